# revision 1
# baseline (speedup 1.0000x reference)
"""MoE layer (top-2 of 8 experts, d_model=2048, d_hid=4096) on 8 trn2 cores.

Strategy: expert-parallel with host-side token dispatch (the all-to-all
equivalent). Core e holds expert e's weights and processes only the tokens
routed to expert e (capacity-padded to C, a multiple of 128). The router
math (logits -> top-2 combine weight) is recomputed on-device per core for
its own tokens; the host's numpy routing is used only to PLACE tokens.
Near-tie top-2 flips between host (fp32) and device (bf16/f32r) routing are
harmless: the combine weight w = p_e / (p_top1 + p_top2) is symmetric in the
top-2 set and continuous in the logits, so a flip at a near-tie perturbs the
output by only the logit-noise magnitude.

Per-core device pipeline:
  router: psum[128tok, 8] = sum_k x[k,tok].T @ rw[k, 8]       (PE)
          s = exp(l_e - m1) / (1 + exp(m2 - m1))              (DVE/ACT)
  L1:     h'[h, tok] = gelu(sum_k w1[k,h].T @ x[k,tok] + b1)  (PE + ACT)
  L2:     y[tok, d] = (sum_h h'[h,tok].T @ w2[h, d] + b2) * s (PE + DVE)

Two variants:
  - bf16 "resident" (default): x and h' stay in SBUF for the whole token
    range; w1/w2 stream from HBM exactly once (~40MB/core) -> compute-bound.
  - f32r "chunked" (MOE_DTYPE=f32r): TF32-class precision (~2e-4 rel err),
    tokens processed in <=512 chunks, weights re-streamed per chunk.
"""
import os
import sys

sys.path.insert(0, "/opt/trn_rl_repo")

import numpy as np
import ml_dtypes

import concourse.bass as bass
import concourse.tile as tile
from concourse import bacc, mybir
from concourse.bass_utils import run_bass_kernel_spmd
from concourse.masks import make_identity

P = 128
D_MODEL = 2048
D_HID = 4096
N_EXP = 8
F32R = mybir.dt.float32r
F32 = mybir.dt.float32
BF16 = mybir.dt.bfloat16
KT1 = D_MODEL // P   # 16 k-tiles in layer 1
KT2 = D_HID // P     # 32 k-tiles in layer 2
HT = D_HID // P      # 32 h-tiles of layer-1 output
DT = D_MODEL // 256  # 8 d-tiles of 256 in layer 2

WDT = BF16 if os.environ.get("MOE_DTYPE", "bf16") == "bf16" else F32R
# largest C whose x + h' residency fits SBUF in bf16
C_RESIDENT_MAX = 1408


def _spans_of(C):
    """Split C (multiple of 128, >=512) into matmul-friendly spans (256..512)."""
    assert C % P == 0 and C >= 512
    out = []
    rem = C
    while rem > 640:
        out.append(512)
        rem -= 512
    if rem == 640:
        out.extend([384, 256])
    else:
        out.append(rem)
    assert sum(out) == C and all(256 <= c <= 512 for c in out)
    return [(sum(out[:i]), c) for i, c in enumerate(out)]


def _declare_io(nc, C, wdt):
    t = {}
    t["xTw"] = nc.dram_tensor("xTw", [D_MODEL, C], wdt, kind="ExternalInput").ap()
    t["w1"] = nc.dram_tensor("w1", [D_MODEL, D_HID], wdt, kind="ExternalInput").ap()
    t["b1"] = nc.dram_tensor("b1", [D_HID], F32, kind="ExternalInput").ap()
    t["w2"] = nc.dram_tensor("w2", [D_HID, D_MODEL], wdt, kind="ExternalInput").ap()
    t["b2"] = nc.dram_tensor("b2", [D_MODEL], F32, kind="ExternalInput").ap()
    t["rw"] = nc.dram_tensor("rw", [D_MODEL, N_EXP], wdt, kind="ExternalInput").ap()
    t["oh"] = nc.dram_tensor("oh", [P, N_EXP], F32, kind="ExternalInput").ap()
    t["y"] = nc.dram_tensor("y", [C, D_MODEL], F32, kind="ExternalOutput").ap()
    return t


def _load_consts(nc, singles, io, wdt):
    rw_sb = singles.tile([P, KT1, N_EXP], wdt)
    rwv = io["rw"].rearrange("(kt p) e -> p kt e", p=P)
    nc.sync.dma_start(out=rw_sb, in_=rwv)
    oh_sb = singles.tile([P, N_EXP], F32)
    nc.sync.dma_start(out=oh_sb, in_=io["oh"])
    b1v = io["b1"].rearrange("(a p) -> p a", p=P)  # [128, HT]
    b1_sb = singles.tile([P, HT], F32)
    nc.sync.dma_start(out=b1_sb, in_=b1v)
    b2_sb = singles.tile([P, D_MODEL], F32)
    b2_bc = bass.AP(tensor=io["b2"].tensor, offset=io["b2"].offset,
                    ap=[[0, P]] + list(io["b2"].ap))
    nc.sync.dma_start(out=b2_sb, in_=b2_bc)
    return rw_sb, oh_sb, b1_sb, b2_sb


def _router_block(nc, pools, xr, rw_sb, oh_sb, s_all, ident, spans, g0):
    """Combine weights for a block of token tiles in one batched chain.

    Matmul with rw stationary (8-col LDWEIGHTS) -> logitsT [8, tok] psum;
    PE-transpose each 128-token tile into a [128, ntt*8] block; then one
    ~12-op DVE/ACT chain computes s = exp(l_e - m1)/(1 + exp(m2 - m1)) for
    all tiles at once into s_all[:, g0:g0+ntt].
    """
    rpool, ps_a, ps_b = pools
    C_blk = sum(cs for _, cs in spans)
    ntt = C_blk // P
    lgT_sb = rpool.tile([8, C_blk], F32, tag="lgT")
    for off, cs in spans:
        lgT_ps = ps_a.tile([8, 512], F32, tag="p1")
        for kt in range(KT1):
            nc.tensor.matmul(lgT_ps[:, :cs], lhsT=rw_sb[:, kt, :],
                             rhs=xr[kt][:, off:off + cs],
                             start=(kt == 0), stop=(kt == KT1 - 1))
        nc.vector.tensor_copy(lgT_sb[:, off:off + cs], lgT_ps[:, :cs])
    pr_all = ps_b.tile([P, ntt * N_EXP], F32, tag="p2")
    for t in range(ntt):
        nc.tensor.transpose(pr_all[:, t * N_EXP:(t + 1) * N_EXP],
                            lgT_sb[:, t * P:(t + 1) * P], ident[0:N_EXP, 0:N_EXP])
    lg = rpool.tile([P, ntt, N_EXP], F32, tag="lg")
    nc.vector.tensor_copy(lg[:], pr_all[:].rearrange("p (t e) -> p t e", e=N_EXP))
    m1 = rpool.tile([P, ntt, 1], F32, tag="m1")
    nc.vector.reduce_max(out=m1[:], in_=lg[:], axis=mybir.AxisListType.X)
    m1b = m1[:, :, 0:1].to_broadcast([P, ntt, N_EXP])
    d8 = rpool.tile([P, ntt, N_EXP], F32, tag="d8")
    nc.vector.tensor_tensor(d8[:], lg[:], m1b, mybir.AluOpType.subtract)
    e1 = rpool.tile([P, ntt, N_EXP], F32, tag="e1")
    nc.scalar.activation(e1[:], d8[:], mybir.ActivationFunctionType.Exp)
    ge = rpool.tile([P, ntt, N_EXP], F32, tag="ge")
    nc.vector.tensor_tensor(ge[:], lg[:], m1b, mybir.AluOpType.is_ge)
    mk = rpool.tile([P, ntt, N_EXP], F32, tag="mk")
    nc.vector.tensor_sub(mk[:], e1[:], ge[:])
    m2 = rpool.tile([P, ntt, 1], F32, tag="m2")
    nc.vector.reduce_max(out=m2[:], in_=mk[:], axis=mybir.AxisListType.X)
    dn = rpool.tile([P, ntt, 1], F32, tag="dn")
    nc.vector.tensor_scalar_add(dn[:], m2[:], 1.0)
    rc = rpool.tile([P, ntt, 1], F32, tag="rc")
    nc.vector.reciprocal(out=rc[:], in_=dn[:])
    ohb = oh_sb[:].rearrange("p (o e) -> p o e", o=1).to_broadcast([P, ntt, N_EXP])
    me = rpool.tile([P, ntt, N_EXP], F32, tag="me")
    nc.vector.tensor_tensor(me[:], e1[:], ohb, mybir.AluOpType.mult)
    ms = rpool.tile([P, ntt, 1], F32, tag="ms")
    nc.vector.reduce_sum(out=ms[:], in_=me[:], axis=mybir.AxisListType.X)
    nc.vector.tensor_mul(s_all[:, g0:g0 + ntt], ms[:, :, 0], rc[:, :, 0])


def build_moe_resident(C, wdt=BF16, reps=1, ablate=()):
    """x and h' SBUF-resident for all C tokens; weights stream exactly once.

    reps>1 wraps the whole body in a hardware loop (timing use only)."""
    spans = _spans_of(C)
    nt = C // P
    nc = bacc.Bacc("TRN2", target_bir_lowering=False, debug=False)
    io = _declare_io(nc, C, wdt)
    from contextlib import nullcontext

    with tile.TileContext(nc) as tc:
        with (
            tc.tile_pool(name="singles", bufs=1) as singles,
            tc.tile_pool(name="xpool", bufs=1) as xpool,
            tc.tile_pool(name="hpool", bufs=1) as hpool,
            tc.tile_pool(name="w1pool", bufs=2) as w1pool,
            tc.tile_pool(name="w2pool", bufs=3) as w2pool,
            tc.tile_pool(name="ypool", bufs=6) as ypool,
            tc.tile_pool(name="rpool", bufs=2) as rpool,
            tc.tile_pool(name="ps1", bufs=4, space="PSUM") as ps1,
            tc.tile_pool(name="ps2", bufs=4, space="PSUM") as ps2,
            tc.For_i(0, reps, 1) if reps > 1 else nullcontext(),
        ):
            rw_sb, oh_sb, b1_sb, b2_sb = _load_consts(nc, singles, io, wdt)
            s_all = singles.tile([P, nt], F32)
            ident = singles.tile([P, P], F32, tag="ident")
            make_identity(nc, ident)

            xr = []
            for kt in range(KT1):
                t = xpool.tile([P, C], wdt, tag=f"x{kt}")
                nc.sync.dma_start(out=t, in_=io["xTw"][kt * P:(kt + 1) * P, :])
                xr.append(t)

            if "router" in ablate:
                nc.vector.memset(s_all[:], 1.0)
            else:
                _router_block(nc, (rpool, ps1, ps2), xr, rw_sb, oh_sb, s_all, ident,
                              spans, 0)

            # layer 1: h'[h, tok] = gelu(w1.T @ x + b1)
            htiles = []
            if "l1" in ablate:
                for ht in range(HT):
                    h_t = hpool.tile([P, C], wdt, tag=f"h{ht}")
                    nc.vector.memset(h_t[:], 0.01)
                    htiles.append(h_t)
            w1v = io["w1"].rearrange("(kt p) h -> p kt h", p=P)  # [128, KT1, D_HID]
            for ht in range(HT if "l1" not in ablate else 0):
                w1t = w1pool.tile([P, KT1, P], wdt, tag="w1")
                nc.sync.dma_start(out=w1t, in_=w1v[:, :, ht * P:(ht + 1) * P])
                h_t = hpool.tile([P, C], wdt, tag=f"h{ht}")
                for off, cs in spans:
                    p1 = ps1.tile([P, 512], F32, tag="p1")
                    for kt in range(KT1):
                        nc.tensor.matmul(p1[:, :cs], lhsT=w1t[:, kt, :],
                                         rhs=xr[kt][:, off:off + cs],
                                         start=(kt == 0), stop=(kt == KT1 - 1))
                    nc.scalar.activation(h_t[:, off:off + cs], p1[:, :cs],
                                         mybir.ActivationFunctionType.Gelu,
                                         bias=b1_sb[:, ht:ht + 1])
                htiles.append(h_t)

            # layer 2: y[tok, d] = (h'.T @ w2 + b2) * s
            w2v = io["w2"].rearrange("(kt p) d -> p kt d", p=P)  # [128, KT2, D_MODEL]
            for dt in range(DT if "l2" not in ablate else 0):
                w2t = w2pool.tile([P, KT2, 256], wdt, tag="w2")
                nc.sync.dma_start(out=w2t, in_=w2v[:, :, dt * 256:(dt + 1) * 256])
                for ti in range(nt):
                    p2 = ps2.tile([P, 256], F32, tag="p2")
                    for kt in range(KT2):
                        nc.tensor.matmul(p2[:],
                                         lhsT=htiles[kt][:, ti * P:(ti + 1) * P],
                                         rhs=w2t[:, kt, :],
                                         start=(kt == 0), stop=(kt == KT2 - 1))
                    yt = ypool.tile([P, 256], F32, tag="y")
                    nc.vector.tensor_add(yt[:], p2[:], b2_sb[:, dt * 256:(dt + 1) * 256])
                    nc.vector.tensor_scalar(yt[:], yt[:], s_all[:, ti:ti + 1], None,
                                            op0=mybir.AluOpType.mult)
                    nc.sync.dma_start(
                        out=io["y"][ti * P:(ti + 1) * P, dt * 256:(dt + 1) * 256],
                        in_=yt[:])
    nc.compile()
    return nc


def build_moe_chunked(C, wdt=F32R, reps=1):
    """Tokens processed in <=512 chunks; weights re-streamed per chunk."""
    spans = _spans_of(C)
    nt = C // P
    nc = bacc.Bacc("TRN2", target_bir_lowering=False, debug=False)
    io = _declare_io(nc, C, wdt)
    from contextlib import nullcontext

    with tile.TileContext(nc) as tc:
        with (
            tc.tile_pool(name="singles", bufs=1) as singles,
            tc.tile_pool(name="xpool", bufs=1) as xpool,
            tc.tile_pool(name="hpool", bufs=1) as hpool,
            tc.tile_pool(name="w1pool", bufs=2) as w1pool,
            tc.tile_pool(name="w2pool", bufs=2) as w2pool,
            tc.tile_pool(name="ypool", bufs=3) as ypool,
            tc.tile_pool(name="rpool", bufs=2) as rpool,
            tc.tile_pool(name="ps1", bufs=4, space="PSUM") as ps1,
            tc.tile_pool(name="ps2", bufs=4, space="PSUM") as ps2,
            tc.For_i(0, reps, 1) if reps > 1 else nullcontext(),
        ):
            rw_sb, oh_sb, b1_sb, b2_sb = _load_consts(nc, singles, io, wdt)
            s_all = singles.tile([P, nt], F32)
            ident = singles.tile([P, P], F32, tag="ident")
            make_identity(nc, ident)
            w1v = io["w1"].rearrange("(kt p) h -> p kt h", p=P)
            w2v = io["w2"].rearrange("(kt p) d -> p kt d", p=P)

            for tok0, cs in spans:
                ntt = cs // P
                xr = []
                for kt in range(KT1):
                    t = xpool.tile([P, 512], wdt, tag=f"x{kt}")
                    nc.sync.dma_start(
                        out=t[:, :cs], in_=io["xTw"][kt * P:(kt + 1) * P, tok0:tok0 + cs])
                    xr.append(t)

                _router_block(nc, (rpool, ps1, ps2), xr, rw_sb, oh_sb, s_all, ident,
                              [(0, cs)], tok0 // P)

                htiles = []
                for ht in range(HT):
                    w1t = w1pool.tile([P, KT1, P], wdt, tag="w1")
                    nc.sync.dma_start(out=w1t, in_=w1v[:, :, ht * P:(ht + 1) * P])
                    p1 = ps1.tile([P, 512], F32, tag="p1")
                    for kt in range(KT1):
                        nc.tensor.matmul(p1[:, :cs], lhsT=w1t[:, kt, :], rhs=xr[kt][:, :cs],
                                         start=(kt == 0), stop=(kt == KT1 - 1))
                    h_t = hpool.tile([P, 512], wdt, tag=f"h{ht}")
                    nc.scalar.activation(h_t[:, :cs], p1[:, :cs],
                                         mybir.ActivationFunctionType.Gelu,
                                         bias=b1_sb[:, ht:ht + 1])
                    htiles.append(h_t)

                for dt in range(DT):
                    w2t = w2pool.tile([P, KT2, 256], wdt, tag="w2")
                    nc.sync.dma_start(out=w2t, in_=w2v[:, :, dt * 256:(dt + 1) * 256])
                    for ti in range(ntt):
                        g = tok0 // P + ti
                        p2 = ps2.tile([P, 256], F32, tag="p2")
                        for kt in range(KT2):
                            nc.tensor.matmul(p2[:],
                                             lhsT=htiles[kt][:, ti * P:(ti + 1) * P],
                                             rhs=w2t[:, kt, :],
                                             start=(kt == 0), stop=(kt == KT2 - 1))
                        yt = ypool.tile([P, 256], F32, tag="y")
                        nc.vector.tensor_add(yt[:], p2[:],
                                             b2_sb[:, dt * 256:(dt + 1) * 256])
                        nc.vector.tensor_scalar(yt[:], yt[:], s_all[:, g:g + 1], None,
                                                op0=mybir.AluOpType.mult)
                        nc.sync.dma_start(
                            out=io["y"][tok0 + ti * P:tok0 + (ti + 1) * P,
                                        dt * 256:(dt + 1) * 256],
                            in_=yt[:])
    nc.compile()
    return nc


def build_moe(C, wdt=None, reps=1):
    wdt = WDT if wdt is None else wdt
    if wdt == BF16 and C <= C_RESIDENT_MAX:
        return build_moe_resident(C, wdt, reps=reps)
    return build_moe_chunked(C, wdt, reps=reps)


def build_null(C, wdt=None):
    """Null kernel with identical I/O signature — dispatch-overhead calibration."""
    wdt = WDT if wdt is None else wdt
    nc = bacc.Bacc("TRN2", target_bir_lowering=False, debug=False)
    io = _declare_io(nc, C, wdt)
    with tile.TileContext(nc) as tc:
        with tc.tile_pool(name="sbuf", bufs=1) as pool:
            t = pool.tile([P, 256], F32)
            nc.sync.dma_start(out=t, in_=io["b2"][0:256].rearrange("(a b) -> a b", a=1)
                              .broadcast(0, P))
            nc.sync.dma_start(out=io["y"][0:P, 0:256], in_=t[:])
    nc.compile()
    return nc


def _route_host(xt, router_w):
    """numpy top-2 routing (placement only; weights recomputed on device)."""
    logits = xt @ router_w
    i1 = np.argmax(logits, axis=1)
    masked = logits.copy()
    masked[np.arange(xt.shape[0]), i1] = -np.inf
    i2 = np.argmax(masked, axis=1)
    return i1, i2


def _build_in_maps(xt, inputs, idx, cnts, C, wdt=None):
    wdt = WDT if wdt is None else wdt
    np_w = ml_dtypes.bfloat16 if wdt == BF16 else np.float32
    D = xt.shape[1]
    in_maps = []
    for e in range(N_EXP):
        xe = np.zeros((D, C), dtype=np.float32)
        xe[:, :cnts[e]] = xt[idx[e]].T
        ohe = np.zeros((P, N_EXP), dtype=np.float32)
        ohe[:, e] = 1.0
        in_maps.append({
            "xTw": xe.astype(np_w),
            "w1": np.ascontiguousarray(inputs["w1"][e]).astype(np_w),
            "b1": np.ascontiguousarray(inputs["b1"][e], dtype=np.float32),
            "w2": np.ascontiguousarray(inputs["w2"][e]).astype(np_w),
            "b2": np.ascontiguousarray(inputs["b2"][e], dtype=np.float32),
            "rw": np.ascontiguousarray(inputs["router_w"]).astype(np_w),
            "oh": ohe,
        })
    return in_maps


_NC_CACHE = {}


def _get_nc(C):
    if C not in _NC_CACHE:
        _NC_CACHE[C] = build_moe(C)
    return _NC_CACHE[C]


def kernel(x, router_w, w1, b1, w2, b2):
    x = np.asarray(x, dtype=np.float32)
    inputs = {"router_w": np.asarray(router_w, dtype=np.float32),
              "w1": np.asarray(w1, dtype=np.float32),
              "b1": np.asarray(b1, dtype=np.float32),
              "w2": np.asarray(w2, dtype=np.float32),
              "b2": np.asarray(b2, dtype=np.float32)}

    Bc, Sc, D = x.shape
    T = Bc * Sc
    xt = np.ascontiguousarray(x.reshape(T, D))

    i1, i2 = _route_host(xt, inputs["router_w"])
    idx = [np.where((i1 == e) | (i2 == e))[0] for e in range(N_EXP)]
    cnts = [len(ix) for ix in idx]
    C = max(512, -(-max(cnts) // P) * P)

    nc = _get_nc(C)
    in_maps = _build_in_maps(xt, inputs, idx, cnts, C)
    res = None
    for attempt in range(3):
        try:
            res = run_bass_kernel_spmd(nc, in_maps, core_ids=list(range(N_EXP)))
            break
        except Exception as ex:  # transient device wedge (NRT_EXEC_UNIT_UNRECOVERABLE)
            if attempt == 2:
                raise
            import time as _time
            print(f"kernel: device execute failed ({ex}); retrying", file=sys.stderr)
            _time.sleep(3)

    out = np.zeros((T, D), dtype=np.float32)
    for e in range(N_EXP):
        ye = res.results[e]["y"]
        out[idx[e]] += ye[:cnts[e]]
    return out.reshape(Bc, Sc, D)



# revision 2
# speedup vs baseline: 1.0902x; 1.0902x over previous
"""MoE layer (top-2 of 8 experts, d_model=2048, d_hid=4096) on 8 trn2 cores.

v3: expert-parallel, host token dispatch, all matmuls fp8e4 DoubleRow
(256-deep contraction per pass at 0.5 cyc/row) with 3-term residual
correction sharing ONE psum accumulation group per output tile:

    A @ B ~= A8 @ B8 + Ar8 @ B8 + A8 @ Br8
    where A8 = e4m3(A*S), Ar8 = e4m3(A*S - A8)   (unscaled residuals)

Scales S are powers of two, folded out in the epilogue. Dropped 2nd-order
term + subnormal residual rounding leave ~1.9e-3 relative error (numpy-
verified; device e4m3 casts are bit-identical to ml_dtypes RNE).

Per-core structure (C = padded max expert count, 32-granular):
  prologue: b1/b2/s loads; x8/xr8 span-chunked loads
  L1 (w1 stationary per h-tile): psum[128h, cs] over 24 DoubleRow passes
      h32 = Gelu(p/SW1 + b1)  [ACT];  h8 = e4m3(h32), hr8 = e4m3(h32-h8) [DVE]
  L2 (w2 stationary per d-tile, TRANSPOSED out): psum[128d, cs] over 48
      passes; y^T = (p/SW2 + b2) * s  [2 DVE ops]; DMA out [d, tok] layout
Host computes routing + combine weights s exactly; host gather transposes.
"""
import os
import sys

sys.path.insert(0, "/opt/trn_rl_repo")

import numpy as np
import ml_dtypes

import concourse.bass as bass
import concourse.tile as tile
from concourse import bacc, mybir
from concourse.bass_utils import run_bass_kernel_spmd

P = 128
D_MODEL = 2048
D_HID = 4096
N_EXP = 8
TOP_K = 2
F32 = mybir.dt.float32
FP8 = mybir.dt.float8e4
E4 = ml_dtypes.float8_e4m3
KT1 = D_MODEL // P   # 16 k-tiles in layer 1
KT2 = D_HID // P     # 32 k-tiles in layer 2
HT = D_HID // P      # 32 h-tiles of layer-1 output
DT = D_MODEL // P    # 16 d-tiles in layer 2 (transposed out)
DR = mybir.MatmulPerfMode.DoubleRow
SW1 = 64.0
SW2 = 64.0


def _spans_of(C, limit=512):
    """Near-uniform 4-granular spans, each <= limit."""
    assert C % 4 == 0
    n = -(-C // limit)
    base = (C // n) // 4 * 4
    rem = (C - base * n) // 4
    sizes = [base + 4 if i < rem else base for i in range(n)]
    assert sum(sizes) == C and all(s <= limit for s in sizes)
    out, off = [], 0
    for s in sizes:
        out.append((off, s))
        off += s
    return out


def build_moe_fp8(C, reps=1, ablate=(), bufs=None):
    spans = _spans_of(C)
    bufs = dict({"ps": 6, "tpool": 3, "w1pool": 3, "w2pool": 2},
                **(bufs or {}))
    nc = bacc.Bacc("TRN2", target_bir_lowering=False, debug=False)
    io = {}
    # pre-tiled layouts (host transposes) so every DMA has >=2KB contiguous
    # runs per partition: x [p, kt, c]; w1 [p, ht, kt, j]; w2 [p, dt, kt, j]
    io["x8"] = nc.dram_tensor("x8", [P, KT1, C], FP8, kind="ExternalInput").ap()
    io["xr8"] = nc.dram_tensor("xr8", [P, KT1, C], FP8, kind="ExternalInput").ap()
    io["w1h"] = nc.dram_tensor("w1h", [P, HT, KT1, P], FP8, kind="ExternalInput").ap()
    io["w1l"] = nc.dram_tensor("w1l", [P, HT, KT1, P], FP8, kind="ExternalInput").ap()
    io["w2h"] = nc.dram_tensor("w2h", [P, DT, KT2, P], FP8, kind="ExternalInput").ap()
    io["w2l"] = nc.dram_tensor("w2l", [P, DT, KT2, P], FP8, kind="ExternalInput").ap()
    io["b1"] = nc.dram_tensor("b1", [D_HID], F32, kind="ExternalInput").ap()
    io["b2"] = nc.dram_tensor("b2", [D_MODEL], F32, kind="ExternalInput").ap()
    io["s"] = nc.dram_tensor("s", [C], F32, kind="ExternalInput").ap()
    io["y"] = nc.dram_tensor("y", [D_MODEL, C], F32, kind="ExternalOutput").ap()

    from contextlib import nullcontext

    with tile.TileContext(nc) as tc:
        with (
            tc.tile_pool(name="singles", bufs=1) as singles,
            tc.tile_pool(name="xpool", bufs=1) as xpool,
            tc.tile_pool(name="hpool", bufs=1) as hpool,
            tc.tile_pool(name="w1pool", bufs=bufs["w1pool"]) as w1pool,
            tc.tile_pool(name="w2pool", bufs=bufs["w2pool"]) as w2pool,
            tc.tile_pool(name="tpool", bufs=bufs["tpool"]) as tpool,
            tc.tile_pool(name="ypool", bufs=4) as ypool,
            tc.tile_pool(name="ps", bufs=bufs["ps"], space="PSUM") as ps,
            tc.For_i(0, reps, 1) if reps > 1 else nullcontext(),
        ):
            w1_tiles = {}

            def load_w1(ht):
                w1ht = w1pool.tile([P, KT1, P], FP8, tag="w1h")
                nc.sync.dma_start(out=w1ht, in_=io["w1h"][:, ht, :, :])
                w1lt = w1pool.tile([P, KT1, P], FP8, tag="w1l")
                nc.sync.dma_start(out=w1lt, in_=io["w1l"][:, ht, :, :])
                w1_tiles[ht] = (w1ht, w1lt)

            x8t = xpool.tile([P, KT1, C], FP8, tag="x8")
            xr8t = xpool.tile([P, KT1, C], FP8, tag="xr8")

            # critical-path DMA order on the SP queue: first weights + first
            # x kt-pair chunks, then the rest; constants and y on ACT queue.
            load_w1(0)
            for kp in range(KT1 // 2):
                nc.sync.dma_start(out=x8t[:, 2 * kp:2 * kp + 2, :],
                                  in_=io["x8"][:, 2 * kp:2 * kp + 2, :])
                nc.sync.dma_start(out=xr8t[:, 2 * kp:2 * kp + 2, :],
                                  in_=io["xr8"][:, 2 * kp:2 * kp + 2, :])
                if kp == 0:
                    load_w1(1)

            b1t = singles.tile([P, HT], F32)
            nc.scalar.dma_start(out=b1t, in_=io["b1"].rearrange("(a p) -> p a", p=P))
            b2t = singles.tile([P, DT], F32)
            nc.scalar.dma_start(out=b2t, in_=io["b2"].rearrange("(a p) -> p a", p=P))
            s_rep = singles.tile([P, C], F32)
            s_bc = bass.AP(tensor=io["s"].tensor, offset=io["s"].offset,
                           ap=[[0, P]] + list(io["s"].ap))
            nc.scalar.dma_start(out=s_rep, in_=s_bc)

            h8 = hpool.tile([P, KT2, C], FP8, tag="h8")
            hr8 = hpool.tile([P, KT2, C], FP8, tag="hr8")

            for ht in range(HT):
                if ht + 2 < HT:
                    load_w1(ht + 2)
                w1ht, w1lt = w1_tiles.pop(ht)
                for si, (off, cs) in enumerate(spans):
                    pm = ps.tile([P, 512], F32, tag="pm")
                    for kp in range(KT1 // 2):
                        nc.tensor.matmul(pm[:, :cs],
                                         lhsT=w1ht[:, 2 * kp:2 * kp + 2, :],
                                         rhs=x8t[:, 2 * kp:2 * kp + 2, off:off + cs],
                                         start=(kp == 0), stop=False,
                                         perf_mode=DR)
                    for kp in range(KT1 // 2):
                        nc.tensor.matmul(pm[:, :cs],
                                         lhsT=w1ht[:, 2 * kp:2 * kp + 2, :],
                                         rhs=xr8t[:, 2 * kp:2 * kp + 2, off:off + cs],
                                         start=False, stop=False,
                                         perf_mode=DR)
                    for kp in range(KT1 // 2):
                        nc.tensor.matmul(pm[:, :cs],
                                         lhsT=w1lt[:, 2 * kp:2 * kp + 2, :],
                                         rhs=x8t[:, 2 * kp:2 * kp + 2, off:off + cs],
                                         start=False, stop=(kp == KT1 // 2 - 1),
                                         perf_mode=DR)
                    if "l1chain" in ablate:
                        nc.vector.tensor_copy(h8[:, ht, off:off + 8], pm[:, 0:8])
                        nc.vector.tensor_copy(hr8[:, ht, off:off + 8], pm[:, 8:16])
                        continue
                    h32 = tpool.tile([P, 512], F32, tag="h32")
                    nc.scalar.activation(h32[:, :cs], pm[:, :cs],
                                         mybir.ActivationFunctionType.Gelu,
                                         bias=b1t[:, ht:ht + 1], scale=1.0 / SW1)
                    nc.vector.tensor_copy(h8[:, ht, off:off + cs], h32[:, :cs])
                    nc.vector.tensor_sub(hr8[:, ht, off:off + cs], h32[:, :cs],
                                         h8[:, ht, off:off + cs])

            for dt in range(DT):
                w2ht = w2pool.tile([P, KT2, P], FP8, tag="w2h")
                nc.sync.dma_start(out=w2ht, in_=io["w2h"][:, dt, :, :])
                w2lt = w2pool.tile([P, KT2, P], FP8, tag="w2l")
                nc.sync.dma_start(out=w2lt, in_=io["w2l"][:, dt, :, :])
                for off, cs in spans:
                    pm = ps.tile([P, 512], F32, tag="pm")
                    for kp in range(KT2 // 2):
                        nc.tensor.matmul(pm[:, :cs],
                                         lhsT=w2ht[:, 2 * kp:2 * kp + 2, :],
                                         rhs=h8[:, 2 * kp:2 * kp + 2, off:off + cs],
                                         start=(kp == 0), stop=False,
                                         perf_mode=DR)
                    for kp in range(KT2 // 2):
                        nc.tensor.matmul(pm[:, :cs],
                                         lhsT=w2ht[:, 2 * kp:2 * kp + 2, :],
                                         rhs=hr8[:, 2 * kp:2 * kp + 2, off:off + cs],
                                         start=False, stop=False,
                                         perf_mode=DR)
                    for kp in range(KT2 // 2):
                        nc.tensor.matmul(pm[:, :cs],
                                         lhsT=w2lt[:, 2 * kp:2 * kp + 2, :],
                                         rhs=h8[:, 2 * kp:2 * kp + 2, off:off + cs],
                                         start=False, stop=(kp == KT2 // 2 - 1),
                                         perf_mode=DR)
                    if "l2chain" in ablate:
                        yt0 = ypool.tile([P, 512], F32, tag="y")
                        nc.vector.tensor_copy(yt0[:, 0:16], pm[:, 0:16])
                        nc.sync.dma_start(
                            out=io["y"][dt * P:(dt + 1) * P, off:off + 16],
                            in_=yt0[:, :16])
                        continue
                    yb = tpool.tile([P, 512], F32, tag="yb")
                    nc.vector.tensor_scalar(yb[:, :cs], pm[:, :cs], 1.0 / SW2,
                                            b2t[:, dt:dt + 1],
                                            op0=mybir.AluOpType.mult,
                                            op1=mybir.AluOpType.add)
                    yt = ypool.tile([P, 512], F32, tag="y")
                    nc.vector.tensor_mul(yt[:, :cs], yb[:, :cs],
                                         s_rep[:, off:off + cs])
                    nc.scalar.dma_start(
                        out=io["y"][dt * P:(dt + 1) * P, off:off + cs],
                        in_=yt[:, :cs])
    nc.compile()
    return nc


def _split8(a, scale):
    """a*scale -> (hi, lo) e4m3 pair, lo = unscaled residual."""
    hi = (a * scale).astype(E4)
    lo = (a * scale - hi.astype(np.float32)).astype(E4)
    return hi, lo


def _pretile_w1(w):
    # [D_MODEL, D_HID] -> [P, HT, KT1, P]
    return np.ascontiguousarray(
        w.reshape(KT1, P, HT, P).transpose(1, 2, 0, 3))


def _pretile_w2(w):
    # [D_HID, D_MODEL] -> [P, DT, KT2, P]
    return np.ascontiguousarray(
        w.reshape(KT2, P, DT, P).transpose(1, 2, 0, 3))


def _pretile_x(x):
    # [D_MODEL, C] -> [P, KT1, C]
    return np.ascontiguousarray(x.reshape(KT1, P, -1).transpose(1, 0, 2))


def _route_host(xt, router_w):
    """fp64 routing: returns (i1, i2, s1, s2) per token."""
    logits = xt.astype(np.float64) @ router_w.astype(np.float64)
    i1 = np.argmax(logits, axis=1)
    masked = logits.copy()
    masked[np.arange(xt.shape[0]), i1] = -np.inf
    i2 = np.argmax(masked, axis=1)
    m = logits.max(axis=1)
    p = np.exp(logits - m[:, None])
    p /= p.sum(axis=1, keepdims=True)
    p1 = p[np.arange(xt.shape[0]), i1]
    p2 = p[np.arange(xt.shape[0]), i2]
    s1 = (p1 / (p1 + p2)).astype(np.float32)
    s2 = (p2 / (p1 + p2)).astype(np.float32)
    return i1, i2, s1, s2


def _build_in_maps(xt, inputs, idx, svals, C):
    in_maps = []
    for e in range(N_EXP):
        cnt = len(idx[e])
        xe = np.zeros((D_MODEL, C), dtype=np.float32)
        xe[:, :cnt] = xt[idx[e]].T
        x8, xr8 = _split8(xe, 1.0)
        w1h, w1l = _split8(np.ascontiguousarray(inputs["w1"][e]), SW1)
        w2h, w2l = _split8(np.ascontiguousarray(inputs["w2"][e]), SW2)
        s = np.zeros(C, dtype=np.float32)
        s[:cnt] = svals[e]
        in_maps.append({
            "x8": _pretile_x(x8), "xr8": _pretile_x(xr8),
            "w1h": _pretile_w1(w1h), "w1l": _pretile_w1(w1l),
            "w2h": _pretile_w2(w2h), "w2l": _pretile_w2(w2l),
            "b1": np.ascontiguousarray(inputs["b1"][e], dtype=np.float32),
            "b2": np.ascontiguousarray(inputs["b2"][e], dtype=np.float32),
            "s": s,
        })
    return in_maps


_NC_CACHE = {}


def _get_nc(C):
    if C not in _NC_CACHE:
        _NC_CACHE[C] = build_moe_fp8(C)
    return _NC_CACHE[C]


def kernel(x, router_w, w1, b1, w2, b2):
    x = np.asarray(x, dtype=np.float32)
    inputs = {"w1": np.asarray(w1, dtype=np.float32),
              "b1": np.asarray(b1, dtype=np.float32),
              "w2": np.asarray(w2, dtype=np.float32),
              "b2": np.asarray(b2, dtype=np.float32)}

    Bc, Sc, D = x.shape
    T = Bc * Sc
    xt = np.ascontiguousarray(x.reshape(T, D))

    i1, i2, s1, s2 = _route_host(xt, np.asarray(router_w, dtype=np.float32))
    idx, svals = [], []
    for e in range(N_EXP):
        m1 = i1 == e
        m2 = i2 == e
        ix = np.where(m1 | m2)[0]
        sv = np.where(m1[ix], s1[ix], s2[ix])
        idx.append(ix)
        svals.append(sv)
    cnts = [len(ix) for ix in idx]
    C = max(512, -(-max(cnts) // 4) * 4)

    nc = _get_nc(C)
    in_maps = _build_in_maps(xt, inputs, idx, svals, C)
    res = None
    for attempt in range(3):
        try:
            res = run_bass_kernel_spmd(nc, in_maps, core_ids=list(range(N_EXP)))
            break
        except Exception as ex:
            if attempt == 2:
                raise
            import time as _time
            print(f"kernel: device execute failed ({ex}); retrying", file=sys.stderr)
            _time.sleep(3)

    out = np.zeros((T, D), dtype=np.float32)
    for e in range(N_EXP):
        ye = res.results[e]["y"]  # [D, C] f32, already *s
        out[idx[e]] += ye[:, :len(idx[e])].T
    return out.reshape(Bc, Sc, D)


# revision 3
# speedup vs baseline: 1.1519x; 1.0566x over previous
"""MoE layer (top-2 of 8 experts, d_model=2048, d_hid=4096) on 8 trn2 cores.

v3: expert-parallel, host token dispatch, all matmuls fp8e4 DoubleRow
(256-deep contraction per pass at 0.5 cyc/row) with 3-term residual
correction sharing ONE psum accumulation group per output tile:

    A @ B ~= A8 @ B8 + Ar8 @ B8 + A8 @ Br8
    where A8 = e4m3(A*S), Ar8 = e4m3(A*S - A8)   (unscaled residuals)

Scales S are powers of two, folded out in the epilogue. Dropped 2nd-order
term + subnormal residual rounding leave ~1.9e-3 relative error (numpy-
verified; device e4m3 casts are bit-identical to ml_dtypes RNE).

Per-core structure (C = padded max expert count, 32-granular):
  prologue: b1/b2/s loads; x8/xr8 span-chunked loads
  L1 (w1 stationary per h-tile): psum[128h, cs] over 24 DoubleRow passes
      h32 = Gelu(p/SW1 + b1)  [ACT];  h8 = e4m3(h32), hr8 = e4m3(h32-h8) [DVE]
  L2 (w2 stationary per d-tile, TRANSPOSED out): psum[128d, cs] over 48
      passes; y^T = (p/SW2 + b2) * s  [2 DVE ops]; DMA out [d, tok] layout
Host computes routing + combine weights s exactly; host gather transposes.
"""
import os
import sys

sys.path.insert(0, "/opt/trn_rl_repo")

import numpy as np
import ml_dtypes

import concourse.bass as bass
import concourse.tile as tile
from concourse import bacc, mybir
from concourse.bass_utils import run_bass_kernel_spmd

P = 128
D_MODEL = 2048
D_HID = 4096
N_EXP = 8
TOP_K = 2
F32 = mybir.dt.float32
FP8 = mybir.dt.float8e4
E4 = ml_dtypes.float8_e4m3
KT1 = D_MODEL // P   # 16 k-tiles in layer 1
KT2 = D_HID // P     # 32 k-tiles in layer 2
HT = D_HID // P      # 32 h-tiles of layer-1 output
DT = D_MODEL // P    # 16 d-tiles in layer 2 (transposed out)
DR = mybir.MatmulPerfMode.DoubleRow
WARMUP = 0
SW1 = 64.0
SW2 = 64.0


def _spans_of(C, limit=512):
    """Near-uniform 4-granular spans, each <= limit."""
    assert C % 4 == 0
    n = -(-C // limit)
    base = (C // n) // 4 * 4
    rem = (C - base * n) // 4
    sizes = [base + 4 if i < rem else base for i in range(n)]
    assert sum(sizes) == C and all(s <= limit for s in sizes)
    out, off = [], 0
    for s in sizes:
        out.append((off, s))
        off += s
    return out


def build_moe_fp8(C, G=0, reps=1, ablate=(), bufs=None):
    """G = token-prefix size computed 1-term (low combine weight); tokens
    [G, C) get the full 3-term treatment."""
    assert G % 4 == 0 and 0 <= G < C
    spans_a = [(off, cs, False) for off, cs in (_spans_of(G) if G else [])]
    spans_b = [(G + off, cs, True) for off, cs in _spans_of(C - G)]
    spans = spans_a + spans_b
    bufs = dict({"ps": 6, "tpool": 3, "w1pool": 3, "w2pool": 2},
                **(bufs or {}))
    nc = bacc.Bacc("TRN2", target_bir_lowering=False, debug=False)
    io = {}
    # pre-tiled layouts (host transposes) so every DMA has >=2KB contiguous
    # runs per partition: x [p, kt, c]; w1 [p, ht, kt, j]; w2 [p, dt, kt, j]
    io["x8"] = nc.dram_tensor("x8", [P, KT1, C], FP8, kind="ExternalInput").ap()
    io["xr8"] = nc.dram_tensor("xr8", [P, KT1, C], FP8, kind="ExternalInput").ap()
    io["w1h"] = nc.dram_tensor("w1h", [P, HT, KT1, P], FP8, kind="ExternalInput").ap()
    io["w1l"] = nc.dram_tensor("w1l", [P, HT, KT1, P], FP8, kind="ExternalInput").ap()
    io["w2h"] = nc.dram_tensor("w2h", [P, DT, KT2, P], FP8, kind="ExternalInput").ap()
    io["w2l"] = nc.dram_tensor("w2l", [P, DT, KT2, P], FP8, kind="ExternalInput").ap()
    io["b1"] = nc.dram_tensor("b1", [D_HID], F32, kind="ExternalInput").ap()
    io["b2"] = nc.dram_tensor("b2", [D_MODEL], F32, kind="ExternalInput").ap()
    io["s"] = nc.dram_tensor("s", [C], F32, kind="ExternalInput").ap()
    io["y"] = nc.dram_tensor("y", [D_MODEL, C], F32, kind="ExternalOutput").ap()

    from contextlib import nullcontext

    with tile.TileContext(nc) as tc:
        with (
            tc.tile_pool(name="singles", bufs=1) as singles,
            tc.tile_pool(name="xpool", bufs=1) as xpool,
            tc.tile_pool(name="hpool", bufs=1) as hpool,
            tc.tile_pool(name="w1pool", bufs=bufs["w1pool"]) as w1pool,
            tc.tile_pool(name="w2pool", bufs=bufs["w2pool"]) as w2pool,
            tc.tile_pool(name="tpool", bufs=bufs["tpool"]) as tpool,
            tc.tile_pool(name="ypool", bufs=4) as ypool,
            tc.tile_pool(name="ps", bufs=bufs["ps"], space="PSUM") as ps,
            tc.For_i(0, reps, 1) if reps > 1 else nullcontext(),
        ):
            w1_tiles = {}

            def load_w1(ht):
                w1ht = w1pool.tile([P, KT1, P], FP8, tag="w1h")
                nc.sync.dma_start(out=w1ht, in_=io["w1h"][:, ht, :, :])
                w1lt = w1pool.tile([P, KT1, P], FP8, tag="w1l")
                nc.sync.dma_start(out=w1lt, in_=io["w1l"][:, ht, :, :])
                w1_tiles[ht] = (w1ht, w1lt)

            x8t = xpool.tile([P, KT1, C], FP8, tag="x8")
            xr8t = xpool.tile([P, KT1, C], FP8, tag="xr8")

            # critical-path DMA order on the SP queue: first weights + first
            # x kt-pair chunks, then the rest; constants and y on ACT queue.
            load_w1(0)
            for kp in range(KT1 // 2):
                nc.sync.dma_start(out=x8t[:, 2 * kp:2 * kp + 2, :],
                                  in_=io["x8"][:, 2 * kp:2 * kp + 2, :])
                nc.sync.dma_start(out=xr8t[:, 2 * kp:2 * kp + 2, :],
                                  in_=io["xr8"][:, 2 * kp:2 * kp + 2, :])
                if kp == 0:
                    load_w1(1)

            w1ht0, w1lt0 = w1_tiles[0]
            for wi in range(WARMUP):
                pj = ps.tile([P, 512], F32, tag="pm")
                nc.tensor.matmul(pj[:, :P], lhsT=w1ht0[:, 0:2, :],
                                 rhs=w1lt0[:, 0:2, 0:P],
                                 start=True, stop=True, perf_mode=DR)

            b1t = singles.tile([P, HT], F32)
            nc.scalar.dma_start(out=b1t, in_=io["b1"].rearrange("(a p) -> p a", p=P))
            b2t = singles.tile([P, DT], F32)
            nc.scalar.dma_start(out=b2t, in_=io["b2"].rearrange("(a p) -> p a", p=P))
            s_rep = singles.tile([P, C], F32)
            s_bc = bass.AP(tensor=io["s"].tensor, offset=io["s"].offset,
                           ap=[[0, P]] + list(io["s"].ap))
            nc.scalar.dma_start(out=s_rep, in_=s_bc)

            h8 = hpool.tile([P, KT2, C], FP8, tag="h8")
            hr8 = hpool.tile([P, KT2, C], FP8, tag="hr8")

            for ht in range(HT):
                if ht + 2 < HT:
                    load_w1(ht + 2)
                w1ht, w1lt = w1_tiles.pop(ht)
                for si, (off, cs, full) in enumerate(spans):
                    pm = ps.tile([P, 512], F32, tag="pm")
                    for kp in range(KT1 // 2):
                        nc.tensor.matmul(pm[:, :cs],
                                         lhsT=w1ht[:, 2 * kp:2 * kp + 2, :],
                                         rhs=x8t[:, 2 * kp:2 * kp + 2, off:off + cs],
                                         start=(kp == 0),
                                         stop=(not full and kp == KT1 // 2 - 1),
                                         perf_mode=DR)
                    if full:
                        for kp in range(KT1 // 2):
                            nc.tensor.matmul(pm[:, :cs],
                                             lhsT=w1ht[:, 2 * kp:2 * kp + 2, :],
                                             rhs=xr8t[:, 2 * kp:2 * kp + 2, off:off + cs],
                                             start=False, stop=False,
                                             perf_mode=DR)
                        for kp in range(KT1 // 2):
                            nc.tensor.matmul(pm[:, :cs],
                                             lhsT=w1lt[:, 2 * kp:2 * kp + 2, :],
                                             rhs=x8t[:, 2 * kp:2 * kp + 2, off:off + cs],
                                             start=False, stop=(kp == KT1 // 2 - 1),
                                             perf_mode=DR)
                    if "l1chain" in ablate:
                        nc.vector.tensor_copy(h8[:, ht, off:off + 8], pm[:, 0:8])
                        nc.vector.tensor_copy(hr8[:, ht, off:off + 8], pm[:, 8:16])
                        continue
                    h32 = tpool.tile([P, 512], F32, tag="h32")
                    nc.scalar.activation(h32[:, :cs], pm[:, :cs],
                                         mybir.ActivationFunctionType.Gelu,
                                         bias=b1t[:, ht:ht + 1], scale=1.0 / SW1)
                    nc.vector.tensor_copy(h8[:, ht, off:off + cs], h32[:, :cs])
                    if full:
                        nc.vector.tensor_sub(hr8[:, ht, off:off + cs], h32[:, :cs],
                                             h8[:, ht, off:off + cs])

            for dt in range(DT):
                w2ht = w2pool.tile([P, KT2, P], FP8, tag="w2h")
                nc.sync.dma_start(out=w2ht, in_=io["w2h"][:, dt, :, :])
                w2lt = w2pool.tile([P, KT2, P], FP8, tag="w2l")
                nc.sync.dma_start(out=w2lt, in_=io["w2l"][:, dt, :, :])
                for off, cs, full in (spans[::-1] if dt == DT - 1 else spans):
                    pm = ps.tile([P, 512], F32, tag="pm")
                    for kp in range(KT2 // 2):
                        nc.tensor.matmul(pm[:, :cs],
                                         lhsT=w2ht[:, 2 * kp:2 * kp + 2, :],
                                         rhs=h8[:, 2 * kp:2 * kp + 2, off:off + cs],
                                         start=(kp == 0),
                                         stop=(not full and kp == KT2 // 2 - 1),
                                         perf_mode=DR)
                    if full:
                        for kp in range(KT2 // 2):
                            nc.tensor.matmul(pm[:, :cs],
                                             lhsT=w2ht[:, 2 * kp:2 * kp + 2, :],
                                             rhs=hr8[:, 2 * kp:2 * kp + 2, off:off + cs],
                                             start=False, stop=False,
                                             perf_mode=DR)
                        for kp in range(KT2 // 2):
                            nc.tensor.matmul(pm[:, :cs],
                                             lhsT=w2lt[:, 2 * kp:2 * kp + 2, :],
                                             rhs=h8[:, 2 * kp:2 * kp + 2, off:off + cs],
                                             start=False, stop=(kp == KT2 // 2 - 1),
                                             perf_mode=DR)
                    if "l2chain" in ablate:
                        yt0 = ypool.tile([P, 512], F32, tag="y")
                        nc.vector.tensor_copy(yt0[:, 0:16], pm[:, 0:16])
                        nc.sync.dma_start(
                            out=io["y"][dt * P:(dt + 1) * P, off:off + 16],
                            in_=yt0[:, :16])
                        continue
                    yb = tpool.tile([P, 512], F32, tag="yb")
                    nc.vector.tensor_scalar(yb[:, :cs], pm[:, :cs], 1.0 / SW2,
                                            b2t[:, dt:dt + 1],
                                            op0=mybir.AluOpType.mult,
                                            op1=mybir.AluOpType.add)
                    yt = ypool.tile([P, 512], F32, tag="y")
                    nc.vector.tensor_mul(yt[:, :cs], yb[:, :cs],
                                         s_rep[:, off:off + cs])
                    nc.scalar.dma_start(
                        out=io["y"][dt * P:(dt + 1) * P, off:off + cs],
                        in_=yt[:, :cs])
    nc.compile()
    return nc


def _split8(a, scale):
    """a*scale -> (hi, lo) e4m3 pair, lo = unscaled residual."""
    hi = (a * scale).astype(E4)
    lo = (a * scale - hi.astype(np.float32)).astype(E4)
    return hi, lo


def _pretile_w1(w):
    # [D_MODEL, D_HID] -> [P, HT, KT1, P]
    return np.ascontiguousarray(
        w.reshape(KT1, P, HT, P).transpose(1, 2, 0, 3))


def _pretile_w2(w):
    # [D_HID, D_MODEL] -> [P, DT, KT2, P]
    return np.ascontiguousarray(
        w.reshape(KT2, P, DT, P).transpose(1, 2, 0, 3))


def _pretile_x(x):
    # [D_MODEL, C] -> [P, KT1, C]
    return np.ascontiguousarray(x.reshape(KT1, P, -1).transpose(1, 0, 2))


def _route_host(xt, router_w):
    """fp64 routing: returns (i1, i2, s1, s2) per token."""
    logits = xt.astype(np.float64) @ router_w.astype(np.float64)
    i1 = np.argmax(logits, axis=1)
    masked = logits.copy()
    masked[np.arange(xt.shape[0]), i1] = -np.inf
    i2 = np.argmax(masked, axis=1)
    m = logits.max(axis=1)
    p = np.exp(logits - m[:, None])
    p /= p.sum(axis=1, keepdims=True)
    p1 = p[np.arange(xt.shape[0]), i1]
    p2 = p[np.arange(xt.shape[0]), i2]
    s1 = (p1 / (p1 + p2)).astype(np.float32)
    s2 = (p2 / (p1 + p2)).astype(np.float32)
    return i1, i2, s1, s2


def _build_in_maps(xt, inputs, idx, svals, C):
    in_maps = []
    for e in range(N_EXP):
        cnt = len(idx[e])
        xe = np.zeros((D_MODEL, C), dtype=np.float32)
        xe[:, :cnt] = xt[idx[e]].T
        x8, xr8 = _split8(xe, 1.0)
        w1h, w1l = _split8(np.ascontiguousarray(inputs["w1"][e]), SW1)
        w2h, w2l = _split8(np.ascontiguousarray(inputs["w2"][e]), SW2)
        s = np.zeros(C, dtype=np.float32)
        s[:cnt] = svals[e]
        in_maps.append({
            "x8": _pretile_x(x8), "xr8": _pretile_x(xr8),
            "w1h": _pretile_w1(w1h), "w1l": _pretile_w1(w1l),
            "w2h": _pretile_w2(w2h), "w2l": _pretile_w2(w2l),
            "b1": np.ascontiguousarray(inputs["b1"][e], dtype=np.float32),
            "b2": np.ascontiguousarray(inputs["b2"][e], dtype=np.float32),
            "s": s,
        })
    return in_maps


_NC_CACHE = {}


def _get_nc(C, G):
    if (C, G) not in _NC_CACHE:
        _NC_CACHE[(C, G)] = build_moe_fp8(C, G)
    return _NC_CACHE[(C, G)]


# combine-weight threshold: expert-paths with s < TAU are computed 1-term
# (error contribution ~1.2e-2 at 0.35 per the numpy study; gate is 2e-2)
TAU = float(os.environ.get("MOE_TAU", "0.40"))


def kernel(x, router_w, w1, b1, w2, b2):
    x = np.asarray(x, dtype=np.float32)
    inputs = {"w1": np.asarray(w1, dtype=np.float32),
              "b1": np.asarray(b1, dtype=np.float32),
              "w2": np.asarray(w2, dtype=np.float32),
              "b2": np.asarray(b2, dtype=np.float32)}

    Bc, Sc, D = x.shape
    T = Bc * Sc
    xt = np.ascontiguousarray(x.reshape(T, D))

    i1, i2, s1, s2 = _route_host(xt, np.asarray(router_w, dtype=np.float32))
    idx, svals = [], []
    for e in range(N_EXP):
        m1 = i1 == e
        m2 = i2 == e
        ix = np.where(m1 | m2)[0]
        sv = np.where(m1[ix], s1[ix], s2[ix])
        order = np.argsort(sv, kind="stable")  # ascending s: 1-term prefix
        idx.append(ix[order])
        svals.append(sv[order])
    cnts = [len(ix) for ix in idx]
    C = max(512, -(-max(cnts) // 4) * 4)
    G = min(int(np.searchsorted(sv, TAU)) for sv in svals) // 4 * 4
    if G < 64:
        G = 0

    nc = _get_nc(C, G)
    in_maps = _build_in_maps(xt, inputs, idx, svals, C)
    res = None
    for attempt in range(3):
        try:
            res = run_bass_kernel_spmd(nc, in_maps, core_ids=list(range(N_EXP)))
            break
        except Exception as ex:
            if attempt == 2:
                raise
            import time as _time
            print(f"kernel: device execute failed ({ex}); retrying", file=sys.stderr)
            _time.sleep(3)

    out = np.zeros((T, D), dtype=np.float32)
    for e in range(N_EXP):
        ye = res.results[e]["y"]  # [D, C] f32, already *s
        out[idx[e]] += ye[:, :len(idx[e])].T
    return out.reshape(Bc, Sc, D)


# revision 4
# speedup vs baseline: 1.1562x; 1.0037x over previous
"""MoE layer (top-2 of 8 experts, d_model=2048, d_hid=4096) on 8 trn2 cores.

v3: expert-parallel, host token dispatch, all matmuls fp8e4 DoubleRow
(256-deep contraction per pass at 0.5 cyc/row) with 3-term residual
correction sharing ONE psum accumulation group per output tile:

    A @ B ~= A8 @ B8 + Ar8 @ B8 + A8 @ Br8
    where A8 = e4m3(A*S), Ar8 = e4m3(A*S - A8)   (unscaled residuals)

Scales S are powers of two, folded out in the epilogue. Dropped 2nd-order
term + subnormal residual rounding leave ~1.9e-3 relative error (numpy-
verified; device e4m3 casts are bit-identical to ml_dtypes RNE).

Per-core structure (C = padded max expert count, 32-granular):
  prologue: b1/b2/s loads; x8/xr8 span-chunked loads
  L1 (w1 stationary per h-tile): psum[128h, cs] over 24 DoubleRow passes
      h32 = Gelu(p/SW1 + b1)  [ACT];  h8 = e4m3(h32), hr8 = e4m3(h32-h8) [DVE]
  L2 (w2 stationary per d-tile, TRANSPOSED out): psum[128d, cs] over 48
      passes; y^T = (p/SW2 + b2) * s  [2 DVE ops]; DMA out [d, tok] layout
Host computes routing + combine weights s exactly; host gather transposes.
"""
import os
import sys

sys.path.insert(0, "/opt/trn_rl_repo")

import numpy as np
import ml_dtypes

import concourse.bass as bass
import concourse.tile as tile
from concourse import bacc, mybir
from concourse.bass_utils import run_bass_kernel_spmd

P = 128
D_MODEL = 2048
D_HID = 4096
N_EXP = 8
TOP_K = 2
F32 = mybir.dt.float32
FP8 = mybir.dt.float8e4
E4 = ml_dtypes.float8_e4m3
KT1 = D_MODEL // P   # 16 k-tiles in layer 1
KT2 = D_HID // P     # 32 k-tiles in layer 2
HT = D_HID // P      # 32 h-tiles of layer-1 output
DT = D_MODEL // P    # 16 d-tiles in layer 2 (transposed out)
DR = mybir.MatmulPerfMode.DoubleRow
WARMUP = 0
SW1 = 64.0
SW2 = 64.0


def _spans_of(C, limit=512):
    """Near-uniform 4-granular spans, each <= limit."""
    assert C % 4 == 0
    n = -(-C // limit)
    base = (C // n) // 4 * 4
    rem = (C - base * n) // 4
    sizes = [base + 4 if i < rem else base for i in range(n)]
    assert sum(sizes) == C and all(s <= limit for s in sizes)
    out, off = [], 0
    for s in sizes:
        out.append((off, s))
        off += s
    return out


def build_moe_fp8(C, G=0, reps=1, ablate=(), bufs=None):
    """G = token-prefix size computed 1-term (low combine weight); tokens
    [G, C) get the full 3-term treatment."""
    assert G % 4 == 0 and 0 <= G < C
    spans_a = [(off, cs, False) for off, cs in (_spans_of(G) if G else [])]
    spans_b = [(G + off, cs, True) for off, cs in _spans_of(C - G)]
    spans = spans_a + spans_b
    bufs = dict({"ps": 6, "tpool": 3, "w1pool": 3, "w2pool": 2},
                **(bufs or {}))
    nc = bacc.Bacc("TRN2", target_bir_lowering=False, debug=False)
    io = {}
    # pre-tiled layouts (host transposes) so every DMA has >=2KB contiguous
    # runs per partition: x [p, kt, c]; w1 [p, ht, kt, j]; w2 [p, dt, kt, j]
    io["x8"] = nc.dram_tensor("x8", [P, KT1, C], FP8, kind="ExternalInput").ap()
    io["xr8"] = nc.dram_tensor("xr8", [P, KT1, C], FP8, kind="ExternalInput").ap()
    io["w1h"] = nc.dram_tensor("w1h", [P, HT, KT1, P], FP8, kind="ExternalInput").ap()
    io["w1l"] = nc.dram_tensor("w1l", [P, HT, KT1, P], FP8, kind="ExternalInput").ap()
    io["w2h"] = nc.dram_tensor("w2h", [P, DT, KT2, P], FP8, kind="ExternalInput").ap()
    io["w2l"] = nc.dram_tensor("w2l", [P, DT, KT2, P], FP8, kind="ExternalInput").ap()
    io["b1"] = nc.dram_tensor("b1", [D_HID], F32, kind="ExternalInput").ap()
    io["b2"] = nc.dram_tensor("b2", [D_MODEL], F32, kind="ExternalInput").ap()
    io["s"] = nc.dram_tensor("s", [C], F32, kind="ExternalInput").ap()
    io["y"] = nc.dram_tensor("y", [D_MODEL, C], F32, kind="ExternalOutput").ap()

    from contextlib import nullcontext

    with tile.TileContext(nc) as tc:
        with (
            tc.tile_pool(name="singles", bufs=1) as singles,
            tc.tile_pool(name="xpool", bufs=1) as xpool,
            tc.tile_pool(name="hpool", bufs=1) as hpool,
            tc.tile_pool(name="w1pool", bufs=bufs["w1pool"]) as w1pool,
            tc.tile_pool(name="w2pool", bufs=bufs["w2pool"]) as w2pool,
            tc.tile_pool(name="tpool", bufs=bufs["tpool"]) as tpool,
            tc.tile_pool(name="ypool", bufs=4) as ypool,
            tc.tile_pool(name="ps", bufs=bufs["ps"], space="PSUM") as ps,
            tc.For_i(0, reps, 1) if reps > 1 else nullcontext(),
        ):
            w1_tiles = {}

            def load_w1(ht):
                w1ht = w1pool.tile([P, KT1, P], FP8, tag="w1h")
                nc.sync.dma_start(out=w1ht, in_=io["w1h"][:, ht, :, :])
                w1lt = w1pool.tile([P, KT1, P], FP8, tag="w1l")
                nc.sync.dma_start(out=w1lt, in_=io["w1l"][:, ht, :, :])
                w1_tiles[ht] = (w1ht, w1lt)

            x8t = xpool.tile([P, KT1, C], FP8, tag="x8")
            xr8t = xpool.tile([P, KT1, C], FP8, tag="xr8")

            # critical-path DMA order on the SP queue: first weights + first
            # x kt-pair chunks, then the rest; constants and y on ACT queue.
            load_w1(0)
            for kp in range(KT1 // 2):
                nc.sync.dma_start(out=x8t[:, 2 * kp:2 * kp + 2, :],
                                  in_=io["x8"][:, 2 * kp:2 * kp + 2, :])
                # 1-term prefix tokens [0, G) never read the x residual
                nc.sync.dma_start(out=xr8t[:, 2 * kp:2 * kp + 2, G:],
                                  in_=io["xr8"][:, 2 * kp:2 * kp + 2, G:])
                if kp == 0:
                    load_w1(1)

            w1ht0, w1lt0 = w1_tiles[0]
            for wi in range(WARMUP):
                pj = ps.tile([P, 512], F32, tag="pm")
                nc.tensor.matmul(pj[:, :P], lhsT=w1ht0[:, 0:2, :],
                                 rhs=w1lt0[:, 0:2, 0:P],
                                 start=True, stop=True, perf_mode=DR)

            b1t = singles.tile([P, HT], F32)
            nc.scalar.dma_start(out=b1t, in_=io["b1"].rearrange("(a p) -> p a", p=P))
            b2t = singles.tile([P, DT], F32)
            nc.scalar.dma_start(out=b2t, in_=io["b2"].rearrange("(a p) -> p a", p=P))
            s_rep = singles.tile([P, C], F32)
            s_bc = bass.AP(tensor=io["s"].tensor, offset=io["s"].offset,
                           ap=[[0, P]] + list(io["s"].ap))
            nc.scalar.dma_start(out=s_rep, in_=s_bc)

            h8 = hpool.tile([P, KT2, C], FP8, tag="h8")
            hr8 = hpool.tile([P, KT2, C], FP8, tag="hr8")

            for ht in range(HT):
                if ht + 2 < HT:
                    load_w1(ht + 2)
                w1ht, w1lt = w1_tiles.pop(ht)
                for si, (off, cs, full) in enumerate(spans):
                    pm = ps.tile([P, 512], F32, tag="pm")
                    for kp in range(KT1 // 2):
                        nc.tensor.matmul(pm[:, :cs],
                                         lhsT=w1ht[:, 2 * kp:2 * kp + 2, :],
                                         rhs=x8t[:, 2 * kp:2 * kp + 2, off:off + cs],
                                         start=(kp == 0),
                                         stop=(not full and kp == KT1 // 2 - 1),
                                         perf_mode=DR)
                    if full:
                        for kp in range(KT1 // 2):
                            nc.tensor.matmul(pm[:, :cs],
                                             lhsT=w1ht[:, 2 * kp:2 * kp + 2, :],
                                             rhs=xr8t[:, 2 * kp:2 * kp + 2, off:off + cs],
                                             start=False, stop=False,
                                             perf_mode=DR)
                        for kp in range(KT1 // 2):
                            nc.tensor.matmul(pm[:, :cs],
                                             lhsT=w1lt[:, 2 * kp:2 * kp + 2, :],
                                             rhs=x8t[:, 2 * kp:2 * kp + 2, off:off + cs],
                                             start=False, stop=(kp == KT1 // 2 - 1),
                                             perf_mode=DR)
                    if "l1chain" in ablate:
                        nc.vector.tensor_copy(h8[:, ht, off:off + 8], pm[:, 0:8])
                        nc.vector.tensor_copy(hr8[:, ht, off:off + 8], pm[:, 8:16])
                        continue
                    h32 = tpool.tile([P, 512], F32, tag="h32")
                    nc.scalar.activation(h32[:, :cs], pm[:, :cs],
                                         mybir.ActivationFunctionType.Gelu,
                                         bias=b1t[:, ht:ht + 1], scale=1.0 / SW1)
                    nc.vector.tensor_copy(h8[:, ht, off:off + cs], h32[:, :cs])
                    if full:
                        nc.vector.tensor_sub(hr8[:, ht, off:off + cs], h32[:, :cs],
                                             h8[:, ht, off:off + cs])

            for dt in range(DT):
                w2ht = w2pool.tile([P, KT2, P], FP8, tag="w2h")
                nc.sync.dma_start(out=w2ht, in_=io["w2h"][:, dt, :, :])
                w2lt = w2pool.tile([P, KT2, P], FP8, tag="w2l")
                nc.sync.dma_start(out=w2lt, in_=io["w2l"][:, dt, :, :])
                for off, cs, full in (spans[::-1] if dt == DT - 1 else spans):
                    pm = ps.tile([P, 512], F32, tag="pm")
                    for kp in range(KT2 // 2):
                        nc.tensor.matmul(pm[:, :cs],
                                         lhsT=w2ht[:, 2 * kp:2 * kp + 2, :],
                                         rhs=h8[:, 2 * kp:2 * kp + 2, off:off + cs],
                                         start=(kp == 0),
                                         stop=(not full and kp == KT2 // 2 - 1),
                                         perf_mode=DR)
                    if full:
                        for kp in range(KT2 // 2):
                            nc.tensor.matmul(pm[:, :cs],
                                             lhsT=w2ht[:, 2 * kp:2 * kp + 2, :],
                                             rhs=hr8[:, 2 * kp:2 * kp + 2, off:off + cs],
                                             start=False, stop=False,
                                             perf_mode=DR)
                        for kp in range(KT2 // 2):
                            nc.tensor.matmul(pm[:, :cs],
                                             lhsT=w2lt[:, 2 * kp:2 * kp + 2, :],
                                             rhs=h8[:, 2 * kp:2 * kp + 2, off:off + cs],
                                             start=False, stop=(kp == KT2 // 2 - 1),
                                             perf_mode=DR)
                    if "l2chain" in ablate:
                        yt0 = ypool.tile([P, 512], F32, tag="y")
                        nc.vector.tensor_copy(yt0[:, 0:16], pm[:, 0:16])
                        nc.sync.dma_start(
                            out=io["y"][dt * P:(dt + 1) * P, off:off + 16],
                            in_=yt0[:, :16])
                        continue
                    yb = tpool.tile([P, 512], F32, tag="yb")
                    nc.vector.tensor_scalar(yb[:, :cs], pm[:, :cs], 1.0 / SW2,
                                            b2t[:, dt:dt + 1],
                                            op0=mybir.AluOpType.mult,
                                            op1=mybir.AluOpType.add)
                    yt = ypool.tile([P, 512], F32, tag="y")
                    nc.vector.tensor_mul(yt[:, :cs], yb[:, :cs],
                                         s_rep[:, off:off + cs])
                    nc.scalar.dma_start(
                        out=io["y"][dt * P:(dt + 1) * P, off:off + cs],
                        in_=yt[:, :cs])
    nc.compile()
    return nc


def _split8(a, scale):
    """a*scale -> (hi, lo) e4m3 pair, lo = unscaled residual."""
    hi = (a * scale).astype(E4)
    lo = (a * scale - hi.astype(np.float32)).astype(E4)
    return hi, lo


def _pretile_w1(w):
    # [D_MODEL, D_HID] -> [P, HT, KT1, P]
    return np.ascontiguousarray(
        w.reshape(KT1, P, HT, P).transpose(1, 2, 0, 3))


def _pretile_w2(w):
    # [D_HID, D_MODEL] -> [P, DT, KT2, P]
    return np.ascontiguousarray(
        w.reshape(KT2, P, DT, P).transpose(1, 2, 0, 3))


def _pretile_x(x):
    # [D_MODEL, C] -> [P, KT1, C]
    return np.ascontiguousarray(x.reshape(KT1, P, -1).transpose(1, 0, 2))


def _route_host(xt, router_w):
    """fp64 routing: returns (i1, i2, s1, s2) per token."""
    logits = xt.astype(np.float64) @ router_w.astype(np.float64)
    i1 = np.argmax(logits, axis=1)
    masked = logits.copy()
    masked[np.arange(xt.shape[0]), i1] = -np.inf
    i2 = np.argmax(masked, axis=1)
    m = logits.max(axis=1)
    p = np.exp(logits - m[:, None])
    p /= p.sum(axis=1, keepdims=True)
    p1 = p[np.arange(xt.shape[0]), i1]
    p2 = p[np.arange(xt.shape[0]), i2]
    s1 = (p1 / (p1 + p2)).astype(np.float32)
    s2 = (p2 / (p1 + p2)).astype(np.float32)
    return i1, i2, s1, s2


def _build_in_maps(xt, inputs, idx, svals, C):
    in_maps = []
    for e in range(N_EXP):
        cnt = len(idx[e])
        xe = np.zeros((D_MODEL, C), dtype=np.float32)
        xe[:, :cnt] = xt[idx[e]].T
        x8, xr8 = _split8(xe, 1.0)
        w1h, w1l = _split8(np.ascontiguousarray(inputs["w1"][e]), SW1)
        w2h, w2l = _split8(np.ascontiguousarray(inputs["w2"][e]), SW2)
        s = np.zeros(C, dtype=np.float32)
        s[:cnt] = svals[e]
        in_maps.append({
            "x8": _pretile_x(x8), "xr8": _pretile_x(xr8),
            "w1h": _pretile_w1(w1h), "w1l": _pretile_w1(w1l),
            "w2h": _pretile_w2(w2h), "w2l": _pretile_w2(w2l),
            "b1": np.ascontiguousarray(inputs["b1"][e], dtype=np.float32),
            "b2": np.ascontiguousarray(inputs["b2"][e], dtype=np.float32),
            "s": s,
        })
    return in_maps


_NC_CACHE = {}


def _get_nc(C, G):
    if (C, G) not in _NC_CACHE:
        _NC_CACHE[(C, G)] = build_moe_fp8(C, G)
    return _NC_CACHE[(C, G)]


# combine-weight threshold: expert-paths with s < TAU are computed 1-term
# (error contribution ~1.2e-2 at 0.35 per the numpy study; gate is 2e-2)
TAU = float(os.environ.get("MOE_TAU", "0.40"))


def kernel(x, router_w, w1, b1, w2, b2):
    x = np.asarray(x, dtype=np.float32)
    inputs = {"w1": np.asarray(w1, dtype=np.float32),
              "b1": np.asarray(b1, dtype=np.float32),
              "w2": np.asarray(w2, dtype=np.float32),
              "b2": np.asarray(b2, dtype=np.float32)}

    Bc, Sc, D = x.shape
    T = Bc * Sc
    xt = np.ascontiguousarray(x.reshape(T, D))

    i1, i2, s1, s2 = _route_host(xt, np.asarray(router_w, dtype=np.float32))
    idx, svals = [], []
    for e in range(N_EXP):
        m1 = i1 == e
        m2 = i2 == e
        ix = np.where(m1 | m2)[0]
        sv = np.where(m1[ix], s1[ix], s2[ix])
        order = np.argsort(sv, kind="stable")  # ascending s: 1-term prefix
        idx.append(ix[order])
        svals.append(sv[order])
    cnts = [len(ix) for ix in idx]
    C = max(512, -(-max(cnts) // 4) * 4)
    G = min(int(np.searchsorted(sv, TAU)) for sv in svals) // 4 * 4
    if G < 64:
        G = 0

    nc = _get_nc(C, G)
    in_maps = _build_in_maps(xt, inputs, idx, svals, C)
    res = None
    for attempt in range(3):
        try:
            res = run_bass_kernel_spmd(nc, in_maps, core_ids=list(range(N_EXP)))
            break
        except Exception as ex:
            if attempt == 2:
                raise
            import time as _time
            print(f"kernel: device execute failed ({ex}); retrying", file=sys.stderr)
            _time.sleep(3)

    out = np.zeros((T, D), dtype=np.float32)
    for e in range(N_EXP):
        ye = res.results[e]["y"]  # [D, C] f32, already *s
        out[idx[e]] += ye[:, :len(idx[e])].T
    return out.reshape(Bc, Sc, D)


# revision 5
# speedup vs baseline: 1.1859x; 1.0257x over previous
"""MoE layer (top-2 of 8 experts, d_model=2048, d_hid=4096) on 8 trn2 cores.

v3: expert-parallel, host token dispatch, all matmuls fp8e4 DoubleRow
(256-deep contraction per pass at 0.5 cyc/row) with 3-term residual
correction sharing ONE psum accumulation group per output tile:

    A @ B ~= A8 @ B8 + Ar8 @ B8 + A8 @ Br8
    where A8 = e4m3(A*S), Ar8 = e4m3(A*S - A8)   (unscaled residuals)

Scales S are powers of two, folded out in the epilogue. Dropped 2nd-order
term + subnormal residual rounding leave ~1.9e-3 relative error (numpy-
verified; device e4m3 casts are bit-identical to ml_dtypes RNE).

Per-core structure (C = padded max expert count, 32-granular):
  prologue: b1/b2/s loads; x8/xr8 span-chunked loads
  L1 (w1 stationary per h-tile): psum[128h, cs] over 24 DoubleRow passes
      h32 = Gelu(p/SW1 + b1)  [ACT];  h8 = e4m3(h32), hr8 = e4m3(h32-h8) [DVE]
  L2 (w2 stationary per d-tile, TRANSPOSED out): psum[128d, cs] over 48
      passes; y^T = (p/SW2 + b2) * s  [2 DVE ops]; DMA out [d, tok] layout
Host computes routing + combine weights s exactly; host gather transposes.
"""
import os
import sys

sys.path.insert(0, "/opt/trn_rl_repo")

import numpy as np
import ml_dtypes

import concourse.bass as bass
import concourse.tile as tile
from concourse import bacc, mybir
from concourse.bass_utils import run_bass_kernel_spmd

P = 128
D_MODEL = 2048
D_HID = 4096
N_EXP = 8
TOP_K = 2
F32 = mybir.dt.float32
FP8 = mybir.dt.float8e4
E4 = ml_dtypes.float8_e4m3
KT1 = D_MODEL // P   # 16 k-tiles in layer 1
KT2 = D_HID // P     # 32 k-tiles in layer 2
HT = D_HID // P      # 32 h-tiles of layer-1 output
DT = D_MODEL // P    # 16 d-tiles in layer 2 (transposed out)
DR = mybir.MatmulPerfMode.DoubleRow
WARMUP = 0
SW1 = 64.0
SW2 = 64.0


def _spans_of(C, limit=512):
    """Near-uniform 4-granular spans, each <= limit."""
    assert C % 4 == 0
    n = -(-C // limit)
    base = (C // n) // 4 * 4
    rem = (C - base * n) // 4
    sizes = [base + 4 if i < rem else base for i in range(n)]
    assert sum(sizes) == C and all(s <= limit for s in sizes)
    out, off = [], 0
    for s in sizes:
        out.append((off, s))
        off += s
    return out


def build_moe_fp8(C, G=0, reps=1, ablate=(), bufs=None):
    """G = token-prefix size computed 1-term (low combine weight); tokens
    [G, C) get the full 3-term treatment."""
    assert G % 4 == 0 and 0 <= G < C
    spans_a = [(off, cs, False) for off, cs in (_spans_of(G) if G else [])]
    spans_b = [(G + off, cs, True) for off, cs in _spans_of(C - G)]
    spans = spans_a + spans_b
    bufs = dict({"ps": 6, "tpool": 3, "w1pool": 3, "w2pool": 2},
                **(bufs or {}))
    nc = bacc.Bacc("TRN2", target_bir_lowering=False, debug=False)
    io = {}
    # pre-tiled layouts (host transposes) so every DMA has >=2KB contiguous
    # runs per partition: x [p, kt, c]; w1 [p, ht, kt, j]; w2 [p, dt, kt, j]
    io["x8"] = nc.dram_tensor("x8", [P, KT1, C], FP8, kind="ExternalInput").ap()
    io["xr8"] = nc.dram_tensor("xr8", [P, KT1, C], FP8, kind="ExternalInput").ap()
    io["w1h"] = nc.dram_tensor("w1h", [P, HT, KT1, P], FP8, kind="ExternalInput").ap()
    io["w1l"] = nc.dram_tensor("w1l", [P, HT, KT1, P], FP8, kind="ExternalInput").ap()
    io["w2h"] = nc.dram_tensor("w2h", [P, DT, KT2, P], FP8, kind="ExternalInput").ap()
    io["w2l"] = nc.dram_tensor("w2l", [P, DT, KT2, P], FP8, kind="ExternalInput").ap()
    io["b1"] = nc.dram_tensor("b1", [D_HID], F32, kind="ExternalInput").ap()
    io["b2"] = nc.dram_tensor("b2", [D_MODEL], F32, kind="ExternalInput").ap()
    io["s"] = nc.dram_tensor("s", [C], F32, kind="ExternalInput").ap()
    io["y"] = nc.dram_tensor("y", [D_MODEL, C], F32, kind="ExternalOutput").ap()

    from contextlib import nullcontext

    with tile.TileContext(nc) as tc:
        with (
            tc.tile_pool(name="singles", bufs=1) as singles,
            tc.tile_pool(name="xpool", bufs=1) as xpool,
            tc.tile_pool(name="hpool", bufs=1) as hpool,
            tc.tile_pool(name="w1pool", bufs=bufs["w1pool"]) as w1pool,
            tc.tile_pool(name="w2pool", bufs=bufs["w2pool"]) as w2pool,
            tc.tile_pool(name="tpool", bufs=bufs["tpool"]) as tpool,
            tc.tile_pool(name="ypool", bufs=4) as ypool,
            tc.tile_pool(name="ps", bufs=bufs["ps"], space="PSUM") as ps,
            tc.For_i(0, reps, 1) if reps > 1 else nullcontext(),
        ):
            w1_tiles = {}

            def load_w1(ht):
                w1ht = w1pool.tile([P, KT1, P], FP8, tag="w1h")
                nc.sync.dma_start(out=w1ht, in_=io["w1h"][:, ht, :, :])
                w1lt = w1pool.tile([P, KT1, P], FP8, tag="w1l")
                nc.sync.dma_start(out=w1lt, in_=io["w1l"][:, ht, :, :])
                w1_tiles[ht] = (w1ht, w1lt)

            x8t = xpool.tile([P, KT1, C], FP8, tag="x8")
            xr8t = xpool.tile([P, KT1, C], FP8, tag="xr8")

            # critical-path DMA order on the SP queue: first weights + first
            # x kt-pair chunks, then the rest; constants and y on ACT queue.
            load_w1(0)
            for kp in range(KT1 // 2):
                nc.sync.dma_start(out=x8t[:, 2 * kp:2 * kp + 2, :],
                                  in_=io["x8"][:, 2 * kp:2 * kp + 2, :])
                # 1-term prefix tokens [0, G) never read the x residual
                nc.sync.dma_start(out=xr8t[:, 2 * kp:2 * kp + 2, G:],
                                  in_=io["xr8"][:, 2 * kp:2 * kp + 2, G:])
                if kp == 0:
                    load_w1(1)

            w1ht0, w1lt0 = w1_tiles[0]
            for wi in range(WARMUP):
                pj = ps.tile([P, 512], F32, tag="pm")
                nc.tensor.matmul(pj[:, :P], lhsT=w1ht0[:, 0:2, :],
                                 rhs=w1lt0[:, 0:2, 0:P],
                                 start=True, stop=True, perf_mode=DR)

            b1t = singles.tile([P, HT], F32)
            nc.scalar.dma_start(out=b1t, in_=io["b1"].rearrange("(a p) -> p a", p=P))
            b2t = singles.tile([P, DT], F32)
            nc.scalar.dma_start(out=b2t, in_=io["b2"].rearrange("(a p) -> p a", p=P))
            s_rep = singles.tile([P, C], F32)
            s_bc = bass.AP(tensor=io["s"].tensor, offset=io["s"].offset,
                           ap=[[0, P]] + list(io["s"].ap))
            nc.scalar.dma_start(out=s_rep, in_=s_bc)

            h8 = hpool.tile([P, KT2, C], FP8, tag="h8")
            hr8 = hpool.tile([P, KT2, C], FP8, tag="hr8")

            for ht in range(HT):
                if ht + 2 < HT:
                    load_w1(ht + 2)
                w1ht, w1lt = w1_tiles.pop(ht)
                for si, (off, cs, full) in enumerate(spans):
                    pm = ps.tile([P, 512], F32, tag="pm")
                    for kp in range(KT1 // 2):
                        nc.tensor.matmul(pm[:, :cs],
                                         lhsT=w1ht[:, 2 * kp:2 * kp + 2, :],
                                         rhs=x8t[:, 2 * kp:2 * kp + 2, off:off + cs],
                                         start=(kp == 0),
                                         stop=(not full and kp == KT1 // 2 - 1),
                                         perf_mode=DR)
                    if full:
                        for kp in range(KT1 // 2):
                            nc.tensor.matmul(pm[:, :cs],
                                             lhsT=w1lt[:, 2 * kp:2 * kp + 2, :],
                                             rhs=x8t[:, 2 * kp:2 * kp + 2, off:off + cs],
                                             start=False, stop=False,
                                             perf_mode=DR)
                        for kp in range(KT1 // 2):
                            nc.tensor.matmul(pm[:, :cs],
                                             lhsT=w1ht[:, 2 * kp:2 * kp + 2, :],
                                             rhs=xr8t[:, 2 * kp:2 * kp + 2, off:off + cs],
                                             start=False, stop=(kp == KT1 // 2 - 1),
                                             perf_mode=DR)
                    if "l1chain" in ablate:
                        nc.vector.tensor_copy(h8[:, ht, off:off + 8], pm[:, 0:8])
                        nc.vector.tensor_copy(hr8[:, ht, off:off + 8], pm[:, 8:16])
                        continue
                    h32 = tpool.tile([P, 512], F32, tag="h32")
                    nc.scalar.activation(h32[:, :cs], pm[:, :cs],
                                         mybir.ActivationFunctionType.Gelu,
                                         bias=b1t[:, ht:ht + 1], scale=1.0 / SW1)
                    nc.vector.tensor_copy(h8[:, ht, off:off + cs], h32[:, :cs])
                    if full:
                        nc.vector.tensor_sub(hr8[:, ht, off:off + cs], h32[:, :cs],
                                             h8[:, ht, off:off + cs])

            for dt in range(DT):
                w2ht = w2pool.tile([P, KT2, P], FP8, tag="w2h")
                nc.sync.dma_start(out=w2ht, in_=io["w2h"][:, dt, :, :])
                w2lt = w2pool.tile([P, KT2, P], FP8, tag="w2l")
                nc.sync.dma_start(out=w2lt, in_=io["w2l"][:, dt, :, :])
                for off, cs, full in (spans[::-1] if dt == DT - 1 else spans):
                    pm = ps.tile([P, 512], F32, tag="pm")
                    for kp in range(KT2 // 2):
                        nc.tensor.matmul(pm[:, :cs],
                                         lhsT=w2ht[:, 2 * kp:2 * kp + 2, :],
                                         rhs=h8[:, 2 * kp:2 * kp + 2, off:off + cs],
                                         start=(kp == 0),
                                         stop=(not full and kp == KT2 // 2 - 1),
                                         perf_mode=DR)
                    if full:
                        for kp in range(KT2 // 2):
                            nc.tensor.matmul(pm[:, :cs],
                                             lhsT=w2lt[:, 2 * kp:2 * kp + 2, :],
                                             rhs=h8[:, 2 * kp:2 * kp + 2, off:off + cs],
                                             start=False, stop=False,
                                             perf_mode=DR)
                        for kp in range(KT2 // 2):
                            nc.tensor.matmul(pm[:, :cs],
                                             lhsT=w2ht[:, 2 * kp:2 * kp + 2, :],
                                             rhs=hr8[:, 2 * kp:2 * kp + 2, off:off + cs],
                                             start=False, stop=(kp == KT2 // 2 - 1),
                                             perf_mode=DR)
                    if "l2chain" in ablate:
                        yt0 = ypool.tile([P, 512], F32, tag="y")
                        nc.vector.tensor_copy(yt0[:, 0:16], pm[:, 0:16])
                        nc.sync.dma_start(
                            out=io["y"][dt * P:(dt + 1) * P, off:off + 16],
                            in_=yt0[:, :16])
                        continue
                    yb = tpool.tile([P, 512], F32, tag="yb")
                    nc.vector.tensor_scalar(yb[:, :cs], pm[:, :cs], 1.0 / SW2,
                                            b2t[:, dt:dt + 1],
                                            op0=mybir.AluOpType.mult,
                                            op1=mybir.AluOpType.add)
                    yt = ypool.tile([P, 512], F32, tag="y")
                    nc.vector.tensor_mul(yt[:, :cs], yb[:, :cs],
                                         s_rep[:, off:off + cs])
                    nc.scalar.dma_start(
                        out=io["y"][dt * P:(dt + 1) * P, off:off + cs],
                        in_=yt[:, :cs])
    nc.compile()
    return nc


def _split8(a, scale):
    """a*scale -> (hi, lo) e4m3 pair, lo = unscaled residual."""
    hi = (a * scale).astype(E4)
    lo = (a * scale - hi.astype(np.float32)).astype(E4)
    return hi, lo


def _pretile_w1(w):
    # [D_MODEL, D_HID] -> [P, HT, KT1, P]
    return np.ascontiguousarray(
        w.reshape(KT1, P, HT, P).transpose(1, 2, 0, 3))


def _pretile_w2(w):
    # [D_HID, D_MODEL] -> [P, DT, KT2, P]
    return np.ascontiguousarray(
        w.reshape(KT2, P, DT, P).transpose(1, 2, 0, 3))


def _pretile_x(x):
    # [D_MODEL, C] -> [P, KT1, C]
    return np.ascontiguousarray(x.reshape(KT1, P, -1).transpose(1, 0, 2))


def _route_host(xt, router_w):
    """fp64 routing: returns (i1, i2, s1, s2) per token."""
    logits = xt.astype(np.float64) @ router_w.astype(np.float64)
    i1 = np.argmax(logits, axis=1)
    masked = logits.copy()
    masked[np.arange(xt.shape[0]), i1] = -np.inf
    i2 = np.argmax(masked, axis=1)
    m = logits.max(axis=1)
    p = np.exp(logits - m[:, None])
    p /= p.sum(axis=1, keepdims=True)
    p1 = p[np.arange(xt.shape[0]), i1]
    p2 = p[np.arange(xt.shape[0]), i2]
    s1 = (p1 / (p1 + p2)).astype(np.float32)
    s2 = (p2 / (p1 + p2)).astype(np.float32)
    return i1, i2, s1, s2


def _build_in_maps(xt, inputs, idx, svals, C):
    in_maps = []
    for e in range(N_EXP):
        cnt = len(idx[e])
        xe = np.zeros((D_MODEL, C), dtype=np.float32)
        xe[:, :cnt] = xt[idx[e]].T
        x8, xr8 = _split8(xe, 1.0)
        w1h, w1l = _split8(np.ascontiguousarray(inputs["w1"][e]), SW1)
        w2h, w2l = _split8(np.ascontiguousarray(inputs["w2"][e]), SW2)
        s = np.zeros(C, dtype=np.float32)
        s[:cnt] = svals[e]
        in_maps.append({
            "x8": _pretile_x(x8), "xr8": _pretile_x(xr8),
            "w1h": _pretile_w1(w1h), "w1l": _pretile_w1(w1l),
            "w2h": _pretile_w2(w2h), "w2l": _pretile_w2(w2l),
            "b1": np.ascontiguousarray(inputs["b1"][e], dtype=np.float32),
            "b2": np.ascontiguousarray(inputs["b2"][e], dtype=np.float32),
            "s": s,
        })
    return in_maps


_NC_CACHE = {}


def _get_nc(C, G):
    if (C, G) not in _NC_CACHE:
        _NC_CACHE[(C, G)] = build_moe_fp8(C, G)
    return _NC_CACHE[(C, G)]


# 1-term group size: the G smallest-combine-weight expert-paths per core are
# computed 1-term. G=260 gives 1.66e-2 predicted rel err (gate 2e-2); the
# s < 0.45 cap bounds the largest 1-term combine weight if routing shifts.
G_TARGET = int(os.environ.get("MOE_G", "260"))
S_CAP = 0.45


def kernel(x, router_w, w1, b1, w2, b2):
    x = np.asarray(x, dtype=np.float32)
    inputs = {"w1": np.asarray(w1, dtype=np.float32),
              "b1": np.asarray(b1, dtype=np.float32),
              "w2": np.asarray(w2, dtype=np.float32),
              "b2": np.asarray(b2, dtype=np.float32)}

    Bc, Sc, D = x.shape
    T = Bc * Sc
    xt = np.ascontiguousarray(x.reshape(T, D))

    i1, i2, s1, s2 = _route_host(xt, np.asarray(router_w, dtype=np.float32))
    idx, svals = [], []
    for e in range(N_EXP):
        m1 = i1 == e
        m2 = i2 == e
        ix = np.where(m1 | m2)[0]
        sv = np.where(m1[ix], s1[ix], s2[ix])
        order = np.argsort(sv, kind="stable")  # ascending s: 1-term prefix
        idx.append(ix[order])
        svals.append(sv[order])
    cnts = [len(ix) for ix in idx]
    C = max(512, -(-max(cnts) // 4) * 4)
    cap = min(int(np.searchsorted(sv, S_CAP)) for sv in svals)
    G = min(G_TARGET, cap) // 4 * 4
    if G < 64:
        G = 0

    nc = _get_nc(C, G)
    in_maps = _build_in_maps(xt, inputs, idx, svals, C)
    res = None
    for attempt in range(3):
        try:
            res = run_bass_kernel_spmd(nc, in_maps, core_ids=list(range(N_EXP)))
            break
        except Exception as ex:
            if attempt == 2:
                raise
            import time as _time
            print(f"kernel: device execute failed ({ex}); retrying", file=sys.stderr)
            _time.sleep(3)

    out = np.zeros((T, D), dtype=np.float32)
    for e in range(N_EXP):
        ye = res.results[e]["y"]  # [D, C] f32, already *s
        out[idx[e]] += ye[:, :len(idx[e])].T
    return out.reshape(Bc, Sc, D)


# revision 6
# speedup vs baseline: 1.2017x; 1.0133x over previous
"""MoE layer (top-2 of 8 experts, d_model=2048, d_hid=4096) on 8 trn2 cores.

v3: expert-parallel, host token dispatch, all matmuls fp8e4 DoubleRow
(256-deep contraction per pass at 0.5 cyc/row) with 3-term residual
correction sharing ONE psum accumulation group per output tile:

    A @ B ~= A8 @ B8 + Ar8 @ B8 + A8 @ Br8
    where A8 = e4m3(A*S), Ar8 = e4m3(A*S - A8)   (unscaled residuals)

Scales S are powers of two, folded out in the epilogue. Dropped 2nd-order
term + subnormal residual rounding leave ~1.9e-3 relative error (numpy-
verified; device e4m3 casts are bit-identical to ml_dtypes RNE).

Per-core structure (C = padded max expert count, 32-granular):
  prologue: b1/b2/s loads; x8/xr8 span-chunked loads
  L1 (w1 stationary per h-tile): psum[128h, cs] over 24 DoubleRow passes
      h32 = Gelu(p/SW1 + b1)  [ACT];  h8 = e4m3(h32), hr8 = e4m3(h32-h8) [DVE]
  L2 (w2 stationary per d-tile, TRANSPOSED out): psum[128d, cs] over 48
      passes; y^T = (p/SW2 + b2) * s  [2 DVE ops]; DMA out [d, tok] layout
Host computes routing + combine weights s exactly; host gather transposes.
"""
import os
import sys

sys.path.insert(0, "/opt/trn_rl_repo")

import numpy as np
import ml_dtypes

import concourse.bass as bass
import concourse.tile as tile
from concourse import bacc, mybir
from concourse.bass_utils import run_bass_kernel_spmd

P = 128
D_MODEL = 2048
D_HID = 4096
N_EXP = 8
TOP_K = 2
F32 = mybir.dt.float32
FP8 = mybir.dt.float8e4
E4 = ml_dtypes.float8_e4m3
KT1 = D_MODEL // P   # 16 k-tiles in layer 1
KT2 = D_HID // P     # 32 k-tiles in layer 2
HT = D_HID // P      # 32 h-tiles of layer-1 output
DT = D_MODEL // P    # 16 d-tiles in layer 2 (transposed out)
DR = mybir.MatmulPerfMode.DoubleRow
WARMUP = 0
SW1 = 64.0
SW2 = 64.0


def _spans_of(C, limit=512):
    """Near-uniform 4-granular spans, each <= limit."""
    assert C % 4 == 0
    n = -(-C // limit)
    base = (C // n) // 4 * 4
    rem = (C - base * n) // 4
    sizes = [base + 4 if i < rem else base for i in range(n)]
    assert sum(sizes) == C and all(s <= limit for s in sizes)
    out, off = [], 0
    for s in sizes:
        out.append((off, s))
        off += s
    return out


def build_moe_fp8(C, G=0, reps=1, ablate=(), bufs=None):
    """G = token-prefix size computed 1-term (low combine weight); tokens
    [G, C) get the full 3-term treatment."""
    assert G % 4 == 0 and 0 <= G < C
    spans_a = [(off, cs, False) for off, cs in (_spans_of(G) if G else [])]
    spans_b = [(G + off, cs, True) for off, cs in _spans_of(C - G)]
    spans = spans_a + spans_b
    bufs = dict({"ps": 6, "tpool": 3, "w1pool": 3, "w2pool": 2},
                **(bufs or {}))
    nc = bacc.Bacc("TRN2", target_bir_lowering=False, debug=False)
    io = {}
    # pre-tiled layouts (host transposes) so every DMA has >=2KB contiguous
    # runs per partition: x [p, kt, c]; w1 [p, ht, kt, j]; w2 [p, dt, kt, j]
    io["x8"] = nc.dram_tensor("x8", [P, KT1, C], FP8, kind="ExternalInput").ap()
    io["xr8"] = nc.dram_tensor("xr8", [P, KT1, C], FP8, kind="ExternalInput").ap()
    io["w1h"] = nc.dram_tensor("w1h", [P, HT, KT1, P], FP8, kind="ExternalInput").ap()
    io["w1l"] = nc.dram_tensor("w1l", [P, HT, KT1, P], FP8, kind="ExternalInput").ap()
    io["w2h"] = nc.dram_tensor("w2h", [P, DT, KT2, P], FP8, kind="ExternalInput").ap()
    io["w2l"] = nc.dram_tensor("w2l", [P, DT, KT2, P], FP8, kind="ExternalInput").ap()
    io["b1"] = nc.dram_tensor("b1", [D_HID], F32, kind="ExternalInput").ap()
    io["b2"] = nc.dram_tensor("b2", [D_MODEL], F32, kind="ExternalInput").ap()
    io["s"] = nc.dram_tensor("s", [C], F32, kind="ExternalInput").ap()
    io["y"] = nc.dram_tensor("y", [D_MODEL, C], F32, kind="ExternalOutput").ap()

    from contextlib import nullcontext

    with tile.TileContext(nc) as tc:
        with (
            tc.tile_pool(name="singles", bufs=1) as singles,
            tc.tile_pool(name="xpool", bufs=1) as xpool,
            tc.tile_pool(name="hpool", bufs=1) as hpool,
            tc.tile_pool(name="w1pool", bufs=bufs["w1pool"]) as w1pool,
            tc.tile_pool(name="w2pool", bufs=bufs["w2pool"]) as w2pool,
            tc.tile_pool(name="tpool", bufs=bufs["tpool"]) as tpool,
            tc.tile_pool(name="ypool", bufs=4) as ypool,
            tc.tile_pool(name="ps", bufs=bufs["ps"], space="PSUM") as ps,
            tc.For_i(0, reps, 1) if reps > 1 else nullcontext(),
        ):
            w1_tiles = {}

            def load_w1(ht):
                w1ht = w1pool.tile([P, KT1, P], FP8, tag="w1h")
                nc.sync.dma_start(out=w1ht, in_=io["w1h"][:, ht, :, :])
                w1lt = w1pool.tile([P, KT1, P], FP8, tag="w1l")
                nc.sync.dma_start(out=w1lt, in_=io["w1l"][:, ht, :, :])
                w1_tiles[ht] = (w1ht, w1lt)

            x8t = xpool.tile([P, KT1, C], FP8, tag="x8")
            xr8t = xpool.tile([P, KT1, C], FP8, tag="xr8")

            # critical-path DMA order on the SP queue: first weights + first
            # x kt-pair chunks, then the rest; constants and y on ACT queue.
            load_w1(0)
            for kp in range(KT1 // 2):
                nc.sync.dma_start(out=x8t[:, 2 * kp:2 * kp + 2, :],
                                  in_=io["x8"][:, 2 * kp:2 * kp + 2, :])
                if kp == 0:
                    load_w1(1)
            for kp in range(KT1 // 2):
                # 1-term prefix tokens [0, G) never read the x residual
                nc.sync.dma_start(out=xr8t[:, 2 * kp:2 * kp + 2, G:],
                                  in_=io["xr8"][:, 2 * kp:2 * kp + 2, G:])

            w1ht0, w1lt0 = w1_tiles[0]
            for wi in range(WARMUP):
                pj = ps.tile([P, 512], F32, tag="pm")
                nc.tensor.matmul(pj[:, :P], lhsT=w1ht0[:, 0:2, :],
                                 rhs=w1lt0[:, 0:2, 0:P],
                                 start=True, stop=True, perf_mode=DR)

            b1t = singles.tile([P, HT], F32)
            nc.scalar.dma_start(out=b1t, in_=io["b1"].rearrange("(a p) -> p a", p=P))
            b2t = singles.tile([P, DT], F32)
            nc.scalar.dma_start(out=b2t, in_=io["b2"].rearrange("(a p) -> p a", p=P))
            s_rep = singles.tile([P, C], F32)
            s_bc = bass.AP(tensor=io["s"].tensor, offset=io["s"].offset,
                           ap=[[0, P]] + list(io["s"].ap))
            nc.scalar.dma_start(out=s_rep, in_=s_bc)

            h8 = hpool.tile([P, KT2, C], FP8, tag="h8")
            hr8 = hpool.tile([P, KT2, C], FP8, tag="hr8")

            for ht in range(HT):
                if ht + 2 < HT:
                    load_w1(ht + 2)
                w1ht, w1lt = w1_tiles.pop(ht)
                for si, (off, cs, full) in enumerate(spans):
                    pm = ps.tile([P, 512], F32, tag="pm")
                    for kp in range(KT1 // 2):
                        nc.tensor.matmul(pm[:, :cs],
                                         lhsT=w1ht[:, 2 * kp:2 * kp + 2, :],
                                         rhs=x8t[:, 2 * kp:2 * kp + 2, off:off + cs],
                                         start=(kp == 0),
                                         stop=(not full and kp == KT1 // 2 - 1),
                                         perf_mode=DR)
                    if full:
                        for kp in range(KT1 // 2):
                            nc.tensor.matmul(pm[:, :cs],
                                             lhsT=w1lt[:, 2 * kp:2 * kp + 2, :],
                                             rhs=x8t[:, 2 * kp:2 * kp + 2, off:off + cs],
                                             start=False, stop=False,
                                             perf_mode=DR)
                        for kp in range(KT1 // 2):
                            nc.tensor.matmul(pm[:, :cs],
                                             lhsT=w1ht[:, 2 * kp:2 * kp + 2, :],
                                             rhs=xr8t[:, 2 * kp:2 * kp + 2, off:off + cs],
                                             start=False, stop=(kp == KT1 // 2 - 1),
                                             perf_mode=DR)
                    if "l1chain" in ablate:
                        nc.vector.tensor_copy(h8[:, ht, off:off + 8], pm[:, 0:8])
                        nc.vector.tensor_copy(hr8[:, ht, off:off + 8], pm[:, 8:16])
                        continue
                    h32 = tpool.tile([P, 512], F32, tag="h32")
                    nc.scalar.activation(h32[:, :cs], pm[:, :cs],
                                         mybir.ActivationFunctionType.Gelu,
                                         bias=b1t[:, ht:ht + 1], scale=1.0 / SW1)
                    nc.vector.tensor_copy(h8[:, ht, off:off + cs], h32[:, :cs])
                    if full:
                        nc.vector.tensor_sub(hr8[:, ht, off:off + cs], h32[:, :cs],
                                             h8[:, ht, off:off + cs])

            for dt in range(DT):
                w2ht = w2pool.tile([P, KT2, P], FP8, tag="w2h")
                nc.sync.dma_start(out=w2ht, in_=io["w2h"][:, dt, :, :])
                w2lt = w2pool.tile([P, KT2, P], FP8, tag="w2l")
                nc.sync.dma_start(out=w2lt, in_=io["w2l"][:, dt, :, :])
                for off, cs, full in (spans[::-1] if dt == DT - 1 else spans):
                    pm = ps.tile([P, 512], F32, tag="pm")
                    for kp in range(KT2 // 2):
                        nc.tensor.matmul(pm[:, :cs],
                                         lhsT=w2ht[:, 2 * kp:2 * kp + 2, :],
                                         rhs=h8[:, 2 * kp:2 * kp + 2, off:off + cs],
                                         start=(kp == 0),
                                         stop=(not full and kp == KT2 // 2 - 1),
                                         perf_mode=DR)
                    if full:
                        for kp in range(KT2 // 2):
                            nc.tensor.matmul(pm[:, :cs],
                                             lhsT=w2lt[:, 2 * kp:2 * kp + 2, :],
                                             rhs=h8[:, 2 * kp:2 * kp + 2, off:off + cs],
                                             start=False, stop=False,
                                             perf_mode=DR)
                        for kp in range(KT2 // 2):
                            nc.tensor.matmul(pm[:, :cs],
                                             lhsT=w2ht[:, 2 * kp:2 * kp + 2, :],
                                             rhs=hr8[:, 2 * kp:2 * kp + 2, off:off + cs],
                                             start=False, stop=(kp == KT2 // 2 - 1),
                                             perf_mode=DR)
                    if "l2chain" in ablate:
                        yt0 = ypool.tile([P, 512], F32, tag="y")
                        nc.vector.tensor_copy(yt0[:, 0:16], pm[:, 0:16])
                        nc.sync.dma_start(
                            out=io["y"][dt * P:(dt + 1) * P, off:off + 16],
                            in_=yt0[:, :16])
                        continue
                    yb = tpool.tile([P, 512], F32, tag="yb")
                    nc.vector.tensor_scalar(yb[:, :cs], pm[:, :cs], 1.0 / SW2,
                                            b2t[:, dt:dt + 1],
                                            op0=mybir.AluOpType.mult,
                                            op1=mybir.AluOpType.add)
                    yt = ypool.tile([P, 512], F32, tag="y")
                    nc.vector.tensor_mul(yt[:, :cs], yb[:, :cs],
                                         s_rep[:, off:off + cs])
                    nc.scalar.dma_start(
                        out=io["y"][dt * P:(dt + 1) * P, off:off + cs],
                        in_=yt[:, :cs])
    nc.compile()
    return nc


def _split8(a, scale):
    """a*scale -> (hi, lo) e4m3 pair, lo = unscaled residual."""
    hi = (a * scale).astype(E4)
    lo = (a * scale - hi.astype(np.float32)).astype(E4)
    return hi, lo


def _pretile_w1(w):
    # [D_MODEL, D_HID] -> [P, HT, KT1, P]
    return np.ascontiguousarray(
        w.reshape(KT1, P, HT, P).transpose(1, 2, 0, 3))


def _pretile_w2(w):
    # [D_HID, D_MODEL] -> [P, DT, KT2, P]
    return np.ascontiguousarray(
        w.reshape(KT2, P, DT, P).transpose(1, 2, 0, 3))


def _pretile_x(x):
    # [D_MODEL, C] -> [P, KT1, C]
    return np.ascontiguousarray(x.reshape(KT1, P, -1).transpose(1, 0, 2))


def _route_host(xt, router_w):
    """fp64 routing: returns (i1, i2, s1, s2) per token."""
    logits = xt.astype(np.float64) @ router_w.astype(np.float64)
    i1 = np.argmax(logits, axis=1)
    masked = logits.copy()
    masked[np.arange(xt.shape[0]), i1] = -np.inf
    i2 = np.argmax(masked, axis=1)
    m = logits.max(axis=1)
    p = np.exp(logits - m[:, None])
    p /= p.sum(axis=1, keepdims=True)
    p1 = p[np.arange(xt.shape[0]), i1]
    p2 = p[np.arange(xt.shape[0]), i2]
    s1 = (p1 / (p1 + p2)).astype(np.float32)
    s2 = (p2 / (p1 + p2)).astype(np.float32)
    return i1, i2, s1, s2


def _build_in_maps(xt, inputs, idx, svals, C):
    in_maps = []
    for e in range(N_EXP):
        cnt = len(idx[e])
        xe = np.zeros((D_MODEL, C), dtype=np.float32)
        xe[:, :cnt] = xt[idx[e]].T
        x8, xr8 = _split8(xe, 1.0)
        w1h, w1l = _split8(np.ascontiguousarray(inputs["w1"][e]), SW1)
        w2h, w2l = _split8(np.ascontiguousarray(inputs["w2"][e]), SW2)
        s = np.zeros(C, dtype=np.float32)
        s[:cnt] = svals[e]
        in_maps.append({
            "x8": _pretile_x(x8), "xr8": _pretile_x(xr8),
            "w1h": _pretile_w1(w1h), "w1l": _pretile_w1(w1l),
            "w2h": _pretile_w2(w2h), "w2l": _pretile_w2(w2l),
            "b1": np.ascontiguousarray(inputs["b1"][e], dtype=np.float32),
            "b2": np.ascontiguousarray(inputs["b2"][e], dtype=np.float32),
            "s": s,
        })
    return in_maps


_NC_CACHE = {}


def _get_nc(C, G):
    if (C, G) not in _NC_CACHE:
        _NC_CACHE[(C, G)] = build_moe_fp8(C, G)
    return _NC_CACHE[(C, G)]


# 1-term group size: the G smallest-combine-weight expert-paths per core are
# computed 1-term. G=260 gives 1.66e-2 predicted rel err (gate 2e-2); the
# s < 0.45 cap bounds the largest 1-term combine weight if routing shifts.
G_TARGET = int(os.environ.get("MOE_G", "260"))
S_CAP = 0.45


def kernel(x, router_w, w1, b1, w2, b2):
    x = np.asarray(x, dtype=np.float32)
    inputs = {"w1": np.asarray(w1, dtype=np.float32),
              "b1": np.asarray(b1, dtype=np.float32),
              "w2": np.asarray(w2, dtype=np.float32),
              "b2": np.asarray(b2, dtype=np.float32)}

    Bc, Sc, D = x.shape
    T = Bc * Sc
    xt = np.ascontiguousarray(x.reshape(T, D))

    i1, i2, s1, s2 = _route_host(xt, np.asarray(router_w, dtype=np.float32))
    idx, svals = [], []
    for e in range(N_EXP):
        m1 = i1 == e
        m2 = i2 == e
        ix = np.where(m1 | m2)[0]
        sv = np.where(m1[ix], s1[ix], s2[ix])
        order = np.argsort(sv, kind="stable")  # ascending s: 1-term prefix
        idx.append(ix[order])
        svals.append(sv[order])
    cnts = [len(ix) for ix in idx]
    C = max(512, -(-max(cnts) // 4) * 4)
    cap = min(int(np.searchsorted(sv, S_CAP)) for sv in svals)
    G = min(G_TARGET, cap) // 4 * 4
    if G < 64:
        G = 0

    nc = _get_nc(C, G)
    in_maps = _build_in_maps(xt, inputs, idx, svals, C)
    res = None
    for attempt in range(3):
        try:
            res = run_bass_kernel_spmd(nc, in_maps, core_ids=list(range(N_EXP)))
            break
        except Exception as ex:
            if attempt == 2:
                raise
            import time as _time
            print(f"kernel: device execute failed ({ex}); retrying", file=sys.stderr)
            _time.sleep(3)

    out = np.zeros((T, D), dtype=np.float32)
    for e in range(N_EXP):
        ye = res.results[e]["y"]  # [D, C] f32, already *s
        out[idx[e]] += ye[:, :len(idx[e])].T
    return out.reshape(Bc, Sc, D)


# revision 7
# speedup vs baseline: 1.2035x; 1.0016x over previous
"""MoE layer (top-2 of 8 experts, d_model=2048, d_hid=4096) on 8 trn2 cores.

v3: expert-parallel, host token dispatch, all matmuls fp8e4 DoubleRow
(256-deep contraction per pass at 0.5 cyc/row) with 3-term residual
correction sharing ONE psum accumulation group per output tile:

    A @ B ~= A8 @ B8 + Ar8 @ B8 + A8 @ Br8
    where A8 = e4m3(A*S), Ar8 = e4m3(A*S - A8)   (unscaled residuals)

Scales S are powers of two, folded out in the epilogue. Dropped 2nd-order
term + subnormal residual rounding leave ~1.9e-3 relative error (numpy-
verified; device e4m3 casts are bit-identical to ml_dtypes RNE).

Per-core structure (C = padded max expert count, 32-granular):
  prologue: b1/b2/s loads; x8/xr8 span-chunked loads
  L1 (w1 stationary per h-tile): psum[128h, cs] over 24 DoubleRow passes
      h32 = Gelu(p/SW1 + b1)  [ACT];  h8 = e4m3(h32), hr8 = e4m3(h32-h8) [DVE]
  L2 (w2 stationary per d-tile, TRANSPOSED out): psum[128d, cs] over 48
      passes; y^T = (p/SW2 + b2) * s  [2 DVE ops]; DMA out [d, tok] layout
Host computes routing + combine weights s exactly; host gather transposes.
"""
import os
import sys

sys.path.insert(0, "/opt/trn_rl_repo")

import numpy as np
import ml_dtypes

import concourse.bass as bass
import concourse.tile as tile
from concourse import bacc, mybir
from concourse.bass_utils import run_bass_kernel_spmd

P = 128
D_MODEL = 2048
D_HID = 4096
N_EXP = 8
TOP_K = 2
F32 = mybir.dt.float32
FP8 = mybir.dt.float8e4
E4 = ml_dtypes.float8_e4m3
KT1 = D_MODEL // P   # 16 k-tiles in layer 1
KT2 = D_HID // P     # 32 k-tiles in layer 2
HT = D_HID // P      # 32 h-tiles of layer-1 output
DT = D_MODEL // P    # 16 d-tiles in layer 2 (transposed out)
DR = mybir.MatmulPerfMode.DoubleRow
WARMUP = 0
SW1 = 64.0
SW2 = 64.0


def _spans_of(C, limit=512):
    """Near-uniform 4-granular spans, each <= limit."""
    assert C % 4 == 0
    n = -(-C // limit)
    base = (C // n) // 4 * 4
    rem = (C - base * n) // 4
    sizes = [base + 4 if i < rem else base for i in range(n)]
    assert sum(sizes) == C and all(s <= limit for s in sizes)
    out, off = [], 0
    for s in sizes:
        out.append((off, s))
        off += s
    return out


def build_moe_fp8(C, G=0, reps=1, ablate=(), bufs=None):
    """G = token-prefix size computed 1-term (low combine weight); tokens
    [G, C) get the full 3-term treatment."""
    assert G % 4 == 0 and 0 <= G < C
    spans_a = [(off, cs, False) for off, cs in (_spans_of(G) if G else [])]
    spans_b = [(G + off, cs, True) for off, cs in _spans_of(C - G)]
    spans = spans_a + spans_b
    bufs = dict({"ps": 6, "tpool": 3, "w1pool": 3, "w2pool": 2},
                **(bufs or {}))
    nc = bacc.Bacc("TRN2", target_bir_lowering=False, debug=False)
    io = {}
    # pre-tiled layouts (host transposes) so every DMA has >=2KB contiguous
    # runs per partition: x [p, kt, c]; w1 [p, ht, kt, j]; w2 [p, dt, kt, j]
    io["x8"] = nc.dram_tensor("x8", [P, KT1, C], FP8, kind="ExternalInput").ap()
    io["xr8"] = nc.dram_tensor("xr8", [P, KT1, C], FP8, kind="ExternalInput").ap()
    io["w1h"] = nc.dram_tensor("w1h", [P, HT, KT1, P], FP8, kind="ExternalInput").ap()
    io["w1l"] = nc.dram_tensor("w1l", [P, HT, KT1, P], FP8, kind="ExternalInput").ap()
    io["w2h"] = nc.dram_tensor("w2h", [P, DT, KT2, P], FP8, kind="ExternalInput").ap()
    io["w2l"] = nc.dram_tensor("w2l", [P, DT, KT2, P], FP8, kind="ExternalInput").ap()
    io["b1"] = nc.dram_tensor("b1", [D_HID], F32, kind="ExternalInput").ap()
    io["b2"] = nc.dram_tensor("b2", [D_MODEL], F32, kind="ExternalInput").ap()
    io["s"] = nc.dram_tensor("s", [C], F32, kind="ExternalInput").ap()
    io["y"] = nc.dram_tensor("y", [D_MODEL, C], F32, kind="ExternalOutput").ap()

    from contextlib import nullcontext

    with tile.TileContext(nc) as tc:
        with (
            tc.tile_pool(name="singles", bufs=1) as singles,
            tc.tile_pool(name="xpool", bufs=1) as xpool,
            tc.tile_pool(name="hpool", bufs=1) as hpool,
            tc.tile_pool(name="w1pool", bufs=bufs["w1pool"]) as w1pool,
            tc.tile_pool(name="w2pool", bufs=bufs["w2pool"]) as w2pool,
            tc.tile_pool(name="tpool", bufs=bufs["tpool"]) as tpool,
            tc.tile_pool(name="ypool", bufs=4) as ypool,
            tc.tile_pool(name="ps", bufs=bufs["ps"], space="PSUM") as ps,
            tc.For_i(0, reps, 1) if reps > 1 else nullcontext(),
        ):
            w1_tiles = {}

            def load_w1h(ht):
                w1ht = w1pool.tile([P, KT1, P], FP8, tag="w1h")
                nc.sync.dma_start(out=w1ht, in_=io["w1h"][:, ht, :, :])
                w1_tiles[ht] = [w1ht, None]

            def load_w1l(ht):
                w1lt = w1pool.tile([P, KT1, P], FP8, tag="w1l")
                nc.sync.dma_start(out=w1lt, in_=io["w1l"][:, ht, :, :])
                w1_tiles[ht][1] = w1lt

            def load_w1(ht):
                load_w1h(ht)
                load_w1l(ht)

            x8t = xpool.tile([P, KT1, C], FP8, tag="x8")
            xr8t = xpool.tile([P, KT1, C], FP8, tag="xr8")

            # critical-path DMA order on the SP queue: the 1-term A-spans and
            # main passes need only w1h + x8, so those stream first; w1l and
            # the x residuals arrive during the A/main work.
            load_w1h(0)
            for kp in range(KT1 // 2):
                nc.sync.dma_start(out=x8t[:, 2 * kp:2 * kp + 2, :],
                                  in_=io["x8"][:, 2 * kp:2 * kp + 2, :])
                if kp == 0:
                    load_w1h(1)
            load_w1l(0)
            load_w1l(1)
            for kp in range(KT1 // 2):
                # 1-term prefix tokens [0, G) never read the x residual
                nc.sync.dma_start(out=xr8t[:, 2 * kp:2 * kp + 2, G:],
                                  in_=io["xr8"][:, 2 * kp:2 * kp + 2, G:])

            w1ht0, w1lt0 = w1_tiles[0]
            for wi in range(WARMUP):
                pj = ps.tile([P, 512], F32, tag="pm")
                nc.tensor.matmul(pj[:, :P], lhsT=w1ht0[:, 0:2, :],
                                 rhs=w1lt0[:, 0:2, 0:P],
                                 start=True, stop=True, perf_mode=DR)

            b1t = singles.tile([P, HT], F32)
            nc.scalar.dma_start(out=b1t, in_=io["b1"].rearrange("(a p) -> p a", p=P))

            h8 = hpool.tile([P, KT2, C], FP8, tag="h8")
            hr8 = hpool.tile([P, KT2, C], FP8, tag="hr8")

            for ht in range(HT):
                if ht + 2 < HT:
                    load_w1(ht + 2)
                w1ht, w1lt = w1_tiles.pop(ht)
                for si, (off, cs, full) in enumerate(spans):
                    pm = ps.tile([P, 512], F32, tag="pm")
                    for kp in range(KT1 // 2):
                        nc.tensor.matmul(pm[:, :cs],
                                         lhsT=w1ht[:, 2 * kp:2 * kp + 2, :],
                                         rhs=x8t[:, 2 * kp:2 * kp + 2, off:off + cs],
                                         start=(kp == 0),
                                         stop=(not full and kp == KT1 // 2 - 1),
                                         perf_mode=DR)
                    if full:
                        for kp in range(KT1 // 2):
                            nc.tensor.matmul(pm[:, :cs],
                                             lhsT=w1lt[:, 2 * kp:2 * kp + 2, :],
                                             rhs=x8t[:, 2 * kp:2 * kp + 2, off:off + cs],
                                             start=False, stop=False,
                                             perf_mode=DR)
                        for kp in range(KT1 // 2):
                            nc.tensor.matmul(pm[:, :cs],
                                             lhsT=w1ht[:, 2 * kp:2 * kp + 2, :],
                                             rhs=xr8t[:, 2 * kp:2 * kp + 2, off:off + cs],
                                             start=False, stop=(kp == KT1 // 2 - 1),
                                             perf_mode=DR)
                    if "l1chain" in ablate:
                        nc.vector.tensor_copy(h8[:, ht, off:off + 8], pm[:, 0:8])
                        nc.vector.tensor_copy(hr8[:, ht, off:off + 8], pm[:, 8:16])
                        continue
                    h32 = tpool.tile([P, 512], F32, tag="h32")
                    nc.scalar.activation(h32[:, :cs], pm[:, :cs],
                                         mybir.ActivationFunctionType.Gelu,
                                         bias=b1t[:, ht:ht + 1], scale=1.0 / SW1)
                    nc.vector.tensor_copy(h8[:, ht, off:off + cs], h32[:, :cs])
                    if full:
                        nc.vector.tensor_sub(hr8[:, ht, off:off + cs], h32[:, :cs],
                                             h8[:, ht, off:off + cs])

            # b2/s are first needed by the L2 epilogue chain; issuing their
            # loads here keeps them out of the prologue's critical DMA wave
            b2t = singles.tile([P, DT], F32)
            nc.scalar.dma_start(out=b2t, in_=io["b2"].rearrange("(a p) -> p a", p=P))
            s_rep = singles.tile([P, C], F32)
            s_bc = bass.AP(tensor=io["s"].tensor, offset=io["s"].offset,
                           ap=[[0, P]] + list(io["s"].ap))
            nc.scalar.dma_start(out=s_rep, in_=s_bc)

            for dt in range(DT):
                w2ht = w2pool.tile([P, KT2, P], FP8, tag="w2h")
                nc.sync.dma_start(out=w2ht, in_=io["w2h"][:, dt, :, :])
                w2lt = w2pool.tile([P, KT2, P], FP8, tag="w2l")
                nc.sync.dma_start(out=w2lt, in_=io["w2l"][:, dt, :, :])
                for off, cs, full in (spans[::-1] if dt == DT - 1 else spans):
                    pm = ps.tile([P, 512], F32, tag="pm")
                    for kp in range(KT2 // 2):
                        nc.tensor.matmul(pm[:, :cs],
                                         lhsT=w2ht[:, 2 * kp:2 * kp + 2, :],
                                         rhs=h8[:, 2 * kp:2 * kp + 2, off:off + cs],
                                         start=(kp == 0),
                                         stop=(not full and kp == KT2 // 2 - 1),
                                         perf_mode=DR)
                    if full:
                        for kp in range(KT2 // 2):
                            nc.tensor.matmul(pm[:, :cs],
                                             lhsT=w2lt[:, 2 * kp:2 * kp + 2, :],
                                             rhs=h8[:, 2 * kp:2 * kp + 2, off:off + cs],
                                             start=False, stop=False,
                                             perf_mode=DR)
                        for kp in range(KT2 // 2):
                            nc.tensor.matmul(pm[:, :cs],
                                             lhsT=w2ht[:, 2 * kp:2 * kp + 2, :],
                                             rhs=hr8[:, 2 * kp:2 * kp + 2, off:off + cs],
                                             start=False, stop=(kp == KT2 // 2 - 1),
                                             perf_mode=DR)
                    if "l2chain" in ablate:
                        yt0 = ypool.tile([P, 512], F32, tag="y")
                        nc.vector.tensor_copy(yt0[:, 0:16], pm[:, 0:16])
                        nc.sync.dma_start(
                            out=io["y"][dt * P:(dt + 1) * P, off:off + 16],
                            in_=yt0[:, :16])
                        continue
                    yb = tpool.tile([P, 512], F32, tag="yb")
                    nc.vector.tensor_scalar(yb[:, :cs], pm[:, :cs], 1.0 / SW2,
                                            b2t[:, dt:dt + 1],
                                            op0=mybir.AluOpType.mult,
                                            op1=mybir.AluOpType.add)
                    yt = ypool.tile([P, 512], F32, tag="y")
                    nc.vector.tensor_mul(yt[:, :cs], yb[:, :cs],
                                         s_rep[:, off:off + cs])
                    nc.scalar.dma_start(
                        out=io["y"][dt * P:(dt + 1) * P, off:off + cs],
                        in_=yt[:, :cs])
    nc.compile()
    return nc


def _split8(a, scale):
    """a*scale -> (hi, lo) e4m3 pair, lo = unscaled residual."""
    hi = (a * scale).astype(E4)
    lo = (a * scale - hi.astype(np.float32)).astype(E4)
    return hi, lo


def _pretile_w1(w):
    # [D_MODEL, D_HID] -> [P, HT, KT1, P]
    return np.ascontiguousarray(
        w.reshape(KT1, P, HT, P).transpose(1, 2, 0, 3))


def _pretile_w2(w):
    # [D_HID, D_MODEL] -> [P, DT, KT2, P]
    return np.ascontiguousarray(
        w.reshape(KT2, P, DT, P).transpose(1, 2, 0, 3))


def _pretile_x(x):
    # [D_MODEL, C] -> [P, KT1, C]
    return np.ascontiguousarray(x.reshape(KT1, P, -1).transpose(1, 0, 2))


def _route_host(xt, router_w):
    """fp64 routing: returns (i1, i2, s1, s2) per token."""
    logits = xt.astype(np.float64) @ router_w.astype(np.float64)
    i1 = np.argmax(logits, axis=1)
    masked = logits.copy()
    masked[np.arange(xt.shape[0]), i1] = -np.inf
    i2 = np.argmax(masked, axis=1)
    m = logits.max(axis=1)
    p = np.exp(logits - m[:, None])
    p /= p.sum(axis=1, keepdims=True)
    p1 = p[np.arange(xt.shape[0]), i1]
    p2 = p[np.arange(xt.shape[0]), i2]
    s1 = (p1 / (p1 + p2)).astype(np.float32)
    s2 = (p2 / (p1 + p2)).astype(np.float32)
    return i1, i2, s1, s2


def _build_in_maps(xt, inputs, idx, svals, C):
    in_maps = []
    for e in range(N_EXP):
        cnt = len(idx[e])
        xe = np.zeros((D_MODEL, C), dtype=np.float32)
        xe[:, :cnt] = xt[idx[e]].T
        x8, xr8 = _split8(xe, 1.0)
        w1h, w1l = _split8(np.ascontiguousarray(inputs["w1"][e]), SW1)
        w2h, w2l = _split8(np.ascontiguousarray(inputs["w2"][e]), SW2)
        s = np.zeros(C, dtype=np.float32)
        s[:cnt] = svals[e]
        in_maps.append({
            "x8": _pretile_x(x8), "xr8": _pretile_x(xr8),
            "w1h": _pretile_w1(w1h), "w1l": _pretile_w1(w1l),
            "w2h": _pretile_w2(w2h), "w2l": _pretile_w2(w2l),
            "b1": np.ascontiguousarray(inputs["b1"][e], dtype=np.float32),
            "b2": np.ascontiguousarray(inputs["b2"][e], dtype=np.float32),
            "s": s,
        })
    return in_maps


_NC_CACHE = {}


def _get_nc(C, G):
    if (C, G) not in _NC_CACHE:
        _NC_CACHE[(C, G)] = build_moe_fp8(C, G)
    return _NC_CACHE[(C, G)]


# 1-term group size: the G smallest-combine-weight expert-paths per core are
# computed 1-term. G=260 gives 1.66e-2 predicted rel err (gate 2e-2); the
# s < 0.45 cap bounds the largest 1-term combine weight if routing shifts.
G_TARGET = int(os.environ.get("MOE_G", "260"))
S_CAP = 0.45


def kernel(x, router_w, w1, b1, w2, b2):
    x = np.asarray(x, dtype=np.float32)
    inputs = {"w1": np.asarray(w1, dtype=np.float32),
              "b1": np.asarray(b1, dtype=np.float32),
              "w2": np.asarray(w2, dtype=np.float32),
              "b2": np.asarray(b2, dtype=np.float32)}

    Bc, Sc, D = x.shape
    T = Bc * Sc
    xt = np.ascontiguousarray(x.reshape(T, D))

    i1, i2, s1, s2 = _route_host(xt, np.asarray(router_w, dtype=np.float32))
    idx, svals = [], []
    for e in range(N_EXP):
        m1 = i1 == e
        m2 = i2 == e
        ix = np.where(m1 | m2)[0]
        sv = np.where(m1[ix], s1[ix], s2[ix])
        order = np.argsort(sv, kind="stable")  # ascending s: 1-term prefix
        idx.append(ix[order])
        svals.append(sv[order])
    cnts = [len(ix) for ix in idx]
    C = max(512, -(-max(cnts) // 4) * 4)
    cap = min(int(np.searchsorted(sv, S_CAP)) for sv in svals)
    G = min(G_TARGET, cap) // 4 * 4
    if G < 64:
        G = 0

    nc = _get_nc(C, G)
    in_maps = _build_in_maps(xt, inputs, idx, svals, C)
    res = None
    for attempt in range(3):
        try:
            res = run_bass_kernel_spmd(nc, in_maps, core_ids=list(range(N_EXP)))
            break
        except Exception as ex:
            if attempt == 2:
                raise
            import time as _time
            print(f"kernel: device execute failed ({ex}); retrying", file=sys.stderr)
            _time.sleep(3)

    out = np.zeros((T, D), dtype=np.float32)
    for e in range(N_EXP):
        ye = res.results[e]["y"]  # [D, C] f32, already *s
        out[idx[e]] += ye[:, :len(idx[e])].T
    return out.reshape(Bc, Sc, D)
